# revision 25
# baseline (speedup 1.0000x reference)
"""Trainium2 Bass kernel for nn_MultiHeadDilatedState (B=4, S=4096, H=768).

Sharding: 8 cores = (batch b in 0..4) x (head-group g in 0..2); each core
runs the head phase (gate matmul + SwiGLU + dilated causal convs + neural
memory + router weighting) for its 6 heads over the full sequence in
feature-major layout, then an 8-core AllToAll re-shards token-parallel:
core j runs the mixing matmuls for token window [512j, 512j+512) of every
batch and outputs token-major.  Host assembles the full output.

SPMD constraint: one program for all cores, but conv dilations differ per
head-group.  Solution: emit the union of both groups' tap ops; each op's
per-partition weight column is zero on cores of the other group.

Self-contained: hardcodes all shapes; builds + compiles once per process.
"""
import numpy as np

DILATIONS = [(1, 2, 4), (1, 1, 1), (4, 8, 16), (8, 16, 32), (32, 64, 128),
             (64, 128, 256), (256, 512, 1024), (1, 100, 200), (1, 500, 1000),
             (1, 1024, 2048), (3, 9, 27), (5, 25, 125)]
MEM_HEADS = (6, 7, 8, 9)
HIDDEN = 768
B, S = 4, 4096
N_CORES = 8
# pair quads chosen to maximize lag collisions (ops merge when k*d equal)
GROUPS = [[0, 1, 2, 3, 6, 8], [10, 11, 4, 5, 7, 9]]
PERM_HEADS = GROUPS[0] + GROUPS[1]
TOK = S // N_CORES   # 512
NB = HIDDEN // 128   # 6
NCK = S // 512       # 8

_CACHE = {}


def _conv_sched():
    """Lag-merged conv schedule with 3-engine column ownership.

    Per (pair, layer): a base op (dst = s1*src + bias, DVE tensor_scalar)
    and one op per DISTINCT lag over the 4 heads (2 groups x 2 slots) of
    that pair, spanning all 128 partitions; per-core weight columns carry
    w (or 0 when that core's head has no tap at this lag).  Each layer's
    dst columns are split [0:b0) DVE-stt | [b0:b1) ScalarE-scale+DVE-add |
    [b1:S) Pool-stt so engines never co-write a range.

    Returns (layers, ncol): layers[p][lay] = dict(scol, bcol, ops=[(lag,
    col, d_hi, s_lo, s_hi, p_lo)]), segment [lag:d_hi) on DVE, [s_lo:s_hi)
    via ScalarE, [p_lo:S) on Pool.
    """
    quads = [[GROUPS[0][2 * p], GROUPS[0][2 * p + 1],
              GROUPS[1][2 * p], GROUPS[1][2 * p + 1]] for p in range(3)]
    lag_sets = []
    total_cols = 0
    for p in range(3):
        per_lay = []
        for lay in range(3):
            lags = sorted({k * DILATIONS[h][lay] for h in quads[p]
                           for k in (1, 2, 3) if k * DILATIONS[h][lay] < S})
            per_lay.append(lags)
            total_cols += sum(S - L for L in lags)
        lag_sets.append(per_lay)

    # engine balance (us/kcol): route A = DVE stt 1.08; route C = ScalarE
    # scale 0.93 + DVE add 0.557; route F = Pool tscal+tt 3.42 (gpsimd can't
    # run stt; pool ops are self-contained scale+add).  Bisect makespan T.
    tot = total_cols / 1000.0
    pre_d, pre_s = 51.0, 50.0
    R_POOL = 3.42

    def feasible(T):
        b_p = T / R_POOL
        c_s = max((T - pre_s) / 0.93, 0.0)
        a_d = tot - b_p - c_s
        if a_d < 0:
            b_p = max(tot - c_s, 0.0)
            a_d = 0.0
        return 1.08 * a_d + 0.557 * c_s + pre_d <= T

    lo, hi = 10.0, 1000.0
    for _ in range(60):
        mid = (lo + hi) / 2
        if feasible(mid):
            hi = mid
        else:
            lo = mid
    T = hi
    b_p = min(T / R_POOL, tot)
    c_s = min(max((T - pre_s) / 0.93, 0.0), tot - b_p)
    f_pool = b_p / tot
    f_scal = c_s / tot

    def find_b(lags, target, from_hi):
        # choose boundary b: if from_hi, sum(S - max(L, b)) = target (pool
        # share); else sum(max(0, b - L)) = target (dve share)
        lo_, hi_ = 0, S
        for _ in range(40):
            m = (lo_ + hi_) // 2
            if from_hi:
                v = sum(S - max(L, m) for L in lags)
                if v > target:
                    lo_ = m + 1
                else:
                    hi_ = m
            else:
                v = sum(max(0, m - L) for L in lags)
                if v < target:
                    lo_ = m + 1
                else:
                    hi_ = m
        return (hi_ // 8) * 8

    layers = []
    col = 0
    for p in range(3):
        per_lay = []
        for lay in range(3):
            lags = lag_sets[p][lay]
            lcols = sum(S - L for L in lags)
            b1 = find_b(lags, f_pool * lcols, True)
            b0 = find_b(lags, (1.0 - f_pool - f_scal) * lcols, False)
            b0 = min(b0, b1)
            scol, bcol = col, col + 1
            col += 2
            ops = []
            for L in lags:
                d_hi = max(L, b0)
                s_lo, s_hi = max(L, b0), max(L, b1)
                p_lo = max(L, b1)
                if S - p_lo < 384:  # pool seg too small: fold into scal seg
                    s_hi = S
                    p_lo = S
                ops.append((L, col, d_hi, s_lo, s_hi, p_lo))
                col += 1
            per_lay.append(dict(scol=scol, bcol=bcol, ops=ops))
        layers.append(per_lay)
    return layers, col


CONV_SCHED, CONV_NCOL = _conv_sched()


def _build_bass(reps=1, ph=6, sim=False):
    import concourse.bacc as bacc
    import concourse.mybir as mybir
    import concourse.tile as tile

    f32 = mybir.dt.float32
    f16 = mybir.dt.float16
    AF = mybir.ActivationFunctionType
    OP = mybir.AluOpType

    nc = bacc.Bacc("TRN2", target_bir_lowering=False, debug=False,
                   num_devices=N_CORES)

    def din(name, shape, dt=f32):
        return nc.dram_tensor(name, shape, dt, kind="ExternalInput").ap()

    xT_d = din("xT", [HIDDEN, S], f16)
    wgT_d = din("wgT", [HIDDEN, HIDDEN], f16)
    rT_d = din("rT", [HIDDEN, 8], f16)
    rb_d = din("rb", [8, 1])
    csc_d = din("conv_sc", [128, CONV_NCOL])
    qbd_d = din("mem_qbd", [128, 128], f16)
    kvg_d = din("mem_kvg", [128, 386], f16)
    gbb_d = din("mem_gb_bc", [128, 2])
    wot_d = din("mem_WoT", [128, 256], f16)
    ones_d = din("ones64", [128, 64])
    eind_d = din("E_ind", [8, 384], f16)
    mgT_d = din("mixgT", [HIDDEN, HIDDEN], f16)
    mgb_d = din("mixgb", [HIDDEN, 1])
    mxT_d = din("mixT", [HIDDEN, HIDDEN], f16)
    mxb_d = din("mixb_bc", [128, HIDDEN])
    y_d = nc.dram_tensor("y", [B * TOK, HIDDEN], f32, kind="ExternalOutput").ap()

    with tile.TileContext(nc) as tc:
        with (
            tc.tile_pool(name="const", bufs=1) as constp,
            tc.tile_pool(name="main", bufs=1) as mainp,
            tc.tile_pool(name="xt", bufs=2) as xtp,
            tc.tile_pool(name="tmp", bufs=3) as tmpp,
            tc.tile_pool(name="ps", bufs=2, space="PSUM") as psp,
            tc.tile_pool(name="dram", bufs=1, space="DRAM") as dramp,
        ):
            # ---------------- resident weights / constants ----------------
            wg_sb = [constp.tile([128, HIDDEN], f16, name=f"wg{i}") for i in range(NB)]
            rT_sb = [constp.tile([128, 8], f16, name=f"rt{i}") for i in range(NB)]
            for i in range(NB):
                nc.sync.dma_start(wg_sb[i][:], wgT_d[128 * i:128 * (i + 1), :])
                nc.sync.dma_start(rT_sb[i][:], rT_d[128 * i:128 * (i + 1), :])
            rb_sb = constp.tile([8, 1], f32, name="rb")
            nc.sync.dma_start(rb_sb[:], rb_d[:])
            csc_sb = constp.tile([128, CONV_NCOL], f32, name="csc")
            nc.sync.dma_start(csc_sb[:], csc_d[:])
            qbd_sb = constp.tile([128, 128], f16, name="qbd")
            nc.sync.dma_start(qbd_sb[:], qbd_d[:])
            kvg_sb = constp.tile([128, 386], f16, name="kvgw")
            nc.sync.dma_start(kvg_sb[:], kvg_d[:])
            gbb_sb = constp.tile([128, 2], f32, name="gbb")
            nc.sync.dma_start(gbb_sb[:], gbb_d[:])
            wot_sb = constp.tile([128, 256], f16, name="wot")
            nc.sync.dma_start(wot_sb[:], wot_d[:])
            ones_sb = constp.tile([128, 64], f32, name="ones")
            nc.sync.dma_start(ones_sb[:], ones_d[:])
            eind_sb = constp.tile([8, 384], f16, name="eind")
            nc.sync.dma_start(eind_sb[:], eind_d[:])
            mgT_sb = [constp.tile([128, HIDDEN], f16, name=f"mg{i}") for i in range(NB)]
            mxT_sb = [constp.tile([128, HIDDEN], f16, name=f"mx{i}") for i in range(NB)]
            for i in range(NB):
                nc.sync.dma_start(mgT_sb[i][:], mgT_d[128 * i:128 * (i + 1), :])
                nc.sync.dma_start(mxT_sb[i][:], mxT_d[128 * i:128 * (i + 1), :])
            mgb_sb = constp.tile([128, NB], f32, name="mgb")
            for i in range(NB):
                nc.sync.dma_start(mgb_sb[:, i:i + 1], mgb_d[128 * i:128 * (i + 1), :])
            mxb_sb = constp.tile([128, HIDDEN], f32, name="mxb")
            nc.sync.dma_start(mxb_sb[:], mxb_d[:])

            # ---------------- persistent state (per rep) ----------------
            for _rep in range(reps):
              xg = [mainp.tile([128, S], f16, name=f"xg{p}", tag=f"xg{p}") for p in range(3)]
              C1 = [mainp.tile([128, S], f16, name=f"c1_{p}", tag=f"c1_{p}") for p in range(3)]
              C2m = mainp.tile([128, S], f16, name="c2m", tag="c2m")
              hw_sb = mainp.tile([8, S], f16, name="hww", tag="hww")
              M_bd = mainp.tile([128, 256], f32, name="Mbd", tag="Mbd")
              nc.vector.memset(M_bd[:], 0.0)

              # ======== Phase 1: gate matmul + SwiGLU + router ========
              with nc.named_scope("ph1_gate"):
               for ck in range(NCK):
                  cs = slice(512 * ck, 512 * (ck + 1))
                  xt = [xtp.tile([128, 512], f16, name=f"xt{i}", tag=f"xt{i}")
                        for i in range(NB)]
                  for i in range(NB):
                      nc.sync.dma_start(xt[i][:], xT_d[128 * i:128 * (i + 1), cs])
                  ps_r = psp.tile([8, 512], f32, name="psr", tag="C")
                  for db in range(NB):
                      nc.tensor.matmul(ps_r[:], rT_sb[db][:], xt[db][:],
                                       start=(db == 0), stop=(db == NB - 1))
                  nc.scalar.activation(hw_sb[:, cs], ps_r[:], AF.Sigmoid,
                                       bias=rb_sb[:, 0:1], scale=1.0)
                  for pb in range(3):
                      ps_a = psp.tile([128, 512], f32, name="psa", tag="A")
                      ps_b = psp.tile([128, 512], f32, name="psb", tag="B")
                      for db in range(NB):
                          nc.tensor.matmul(
                              ps_a[:], wg_sb[db][:, 128 * pb:128 * (pb + 1)],
                              xt[db][:], start=(db == 0), stop=(db == NB - 1))
                      for db in range(NB):
                          nc.tensor.matmul(
                              ps_b[:],
                              wg_sb[db][:, 384 + 128 * pb:384 + 128 * (pb + 1)],
                              xt[db][:], start=(db == 0), stop=(db == NB - 1))
                      sig = tmpp.tile([128, 512], f32, name="sig", tag="sig")
                      nc.scalar.activation(sig[:], ps_b[:], AF.Sigmoid)
                      nc.vector.tensor_tensor(xg[pb][:, cs], ps_a[:], sig[:], OP.mult)

              # ======== Phase 2+4: neural memory (pair 2 heads) ========
              # All operands at partition base 0 (HW matmul/engine constraint).
              x_mem = xg[2]
              rd_ck = [mainp.tile([128, 512], f16, name=f"rdck{h}", tag=f"rdck{h}") for h in range(2)]
              mem_o = mainp.tile([128, S], f16, name="memo", tag="memo")
              M_a = mainp.tile([64, 128], f32, name="Ma", tag="Ma")
              M_b = mainp.tile([64, 128], f32, name="Mb", tag="Mb")
              nc.vector.memset(M_a[:], 0.0)
              nc.vector.memset(M_b[:], 0.0)
              _s2 = nc.enter_named_scope("ph2_mem", False)[0]
              for blk in range(S // 128):
                  bs = slice(128 * blk, 128 * (blk + 1))
                  # q projection [d(64), t(128)] per head, both at base 0
                  ps_qa = psp.tile([64, 128], f32, name="psqa", tag="C")
                  ps_qb = psp.tile([64, 128], f32, name="psqb", tag="D", bufs=1)
                  nc.tensor.matmul(ps_qa[:], qbd_sb[:, 0:64], x_mem[:, bs],
                                   start=True, stop=True)
                  nc.tensor.matmul(ps_qb[:], qbd_sb[:, 64:128], x_mem[:, bs],
                                   start=True, stop=True)
                  q_a = tmpp.tile([64, 128], f32, name="qa", tag="qa")
                  q_b = tmpp.tile([64, 128], f32, name="qb", tag="qb")
                  nc.scalar.copy(q_a[:], ps_qa[:])
                  nc.scalar.copy(q_b[:], ps_qb[:])
                  ps_rd = psp.tile([128, 256], f32, name="psrd", tag="B")
                  for half in range(2):
                      c64 = slice(128 * blk + 64 * half, 128 * blk + 64 * (half + 1))
                      # k|v|g projection for this 64-token chunk, token-major
                      ps_kvg = psp.tile([64, 386], f32, name="pskvg", tag="A")
                      nc.tensor.matmul(ps_kvg[:], x_mem[:, c64], kvg_sb[:],
                                       start=True, stop=True)
                      g_sb = tmpp.tile([64, 2], f32, name="gsb", tag="gsb")
                      for hh in range(2):
                          nc.scalar.activation(g_sb[:, hh:hh + 1],
                                               ps_kvg[:, 384 + hh:385 + hh],
                                               AF.Sigmoid,
                                               bias=gbb_sb[0:64, hh:hh + 1],
                                               scale=1.0)
                      kg_sb = tmpp.tile([64, 128], f16, name="kgsb", tag="kgsb")
                      for hh in range(2):
                          nc.vector.tensor_scalar(
                              kg_sb[:, 64 * hh:64 * (hh + 1)],
                              ps_kvg[:, 64 * hh:64 * (hh + 1)],
                              g_sb[:, hh:hh + 1], None, OP.mult)
                      v_sb = tmpp.tile([64, 256], f16, name="vsb", tag="vsb")
                      nc.scalar.copy(v_sb[:], ps_kvg[:, 128:384])
                      # reads (old M): readsT[m, t]; head hh in cols 128*hh+...
                      nc.tensor.matmul(ps_rd[:, 128 * 0 + 64 * half:128 * 0 + 64 * (half + 1)],
                                       M_a[:], q_a[:, 64 * half:64 * (half + 1)],
                                       start=True, stop=True)
                      nc.tensor.matmul(ps_rd[:, 128 * 1 + 64 * half:128 * 1 + 64 * (half + 1)],
                                       M_b[:], q_b[:, 64 * half:64 * (half + 1)],
                                       start=True, stop=True)
                      # decay = 1 - mean(g): one matmul, avgs replicated
                      ps_g = psp.tile([64, 2], f32, name="psg", tag="D", bufs=1)
                      nc.tensor.matmul(ps_g[:], ones_sb[0:64, :], g_sb[:],
                                       start=True, stop=True)
                      decay = tmpp.tile([64, 2], f32, name="decay", tag="decay")
                      nc.scalar.activation(decay[:], ps_g[:], AF.Identity,
                                           bias=1.0, scale=-1.0)
                      # write outer products, per head (base 0)
                      ps_w = psp.tile([64, 256], f32, name="psw", tag="E", bufs=1)
                      nc.tensor.matmul(ps_w[:, 0:128], kg_sb[:, 0:64],
                                       v_sb[:, 0:128], start=True, stop=True)
                      nc.tensor.matmul(ps_w[:, 128:256], kg_sb[:, 64:128],
                                       v_sb[:, 128:256], start=True, stop=True)
                      # M = decay*M + W
                      nc.vector.scalar_tensor_tensor(
                          M_a[:], M_a[:], decay[:, 0:1], ps_w[:, 0:128],
                          OP.mult, OP.add)
                      nc.vector.scalar_tensor_tensor(
                          M_b[:], M_b[:], decay[:, 1:2], ps_w[:, 128:256],
                          OP.mult, OP.add)
                  # evict reads into per-head chunk tiles
                  cc = 128 * blk % 512
                  for hh in range(2):
                      nc.scalar.copy(rd_ck[hh][:, cc:cc + 128],
                                     ps_rd[:, 128 * hh:128 * (hh + 1)])
                  # every 4 blocks: Wout matmuls accumulate stacked [128, 512]
                  if blk % 4 == 3:
                      ck4 = blk // 4
                      cs4 = slice(512 * ck4, 512 * (ck4 + 1))
                      ps_o = psp.tile([128, 512], f32, name="pso", tag="C")
                      nc.tensor.matmul(ps_o[:], wot_sb[:, 0:128], rd_ck[0][:],
                                       start=True, stop=False)
                      nc.tensor.matmul(ps_o[:], wot_sb[:, 128:256], rd_ck[1][:],
                                       start=False, stop=True)
                      nc.scalar.copy(mem_o[:, cs4], ps_o[:])
              nc.leave_named_scope("ph2_mem", _s2, False)

              if ph < 3:
                  nc.sync.dma_start(y_d[0:128, :], mxb_sb[:])
                  continue
              # ======== Phase 3: dilated conv chains (union emission) ========
              # pairs 0,1 ping-pong xg<->C1 (xg is free after layer 0 reads);
              # pair 2 keeps xg intact (scan input): xg->C1->C2m->C1.
              def chain_tiles(p):
                  if p < 2:
                      return [(xg[p], C1[p]), (C1[p], xg[p]), (xg[p], C1[p])]
                  return [(xg[2], C1[2]), (C1[2], C2m), (C2m, C1[2])]

              with nc.named_scope("ph3_conv"):
               max_sw = max((o[4] - o[3] for pl in CONV_SCHED for sch in pl
                             for o in sch["ops"]), default=0)
               max_pw = max((S - o[5] for pl in CONV_SCHED for sch in pl
                             for o in sch["ops"]), default=0)
               tctr = pctr = 0
               for lay in range(3):
                  for p in range(3):
                      src, dst = chain_tiles(p)[lay]
                      sch = CONV_SCHED[p][lay]
                      nc.vector.tensor_scalar(
                          dst[:], src[:], csc_sb[:, sch["scol"]:sch["scol"] + 1],
                          csc_sb[:, sch["bcol"]:sch["bcol"] + 1], OP.mult, OP.add)
                      for (L, cw, d_hi, s_lo, s_hi, p_lo) in sch["ops"]:
                          wcol = csc_sb[:, cw:cw + 1]
                          if S > p_lo:
                              pw = S - p_lo
                              ptmp = tmpp.tile([128, max_pw], f16, name="pt",
                                               tag=f"pt{pctr % 2}", bufs=1)
                              pctr += 1
                              nc.gpsimd.tensor_scalar(
                                  ptmp[:, 0:pw], src[:, p_lo - L:S - L], wcol,
                                  None, OP.mult)
                              nc.gpsimd.tensor_tensor(
                                  dst[:, p_lo:S], dst[:, p_lo:S],
                                  ptmp[:, 0:pw], OP.add)
                          tmp, w = None, 0
                          if s_hi > s_lo:
                              w = s_hi - s_lo
                              tmp = tmpp.tile([128, max_sw], f16, name="ct",
                                              tag=f"ct{tctr % 3}", bufs=1)
                              tctr += 1
                              nc.scalar.activation(tmp[:, 0:w],
                                                   src[:, s_lo - L:s_hi - L],
                                                   AF.Identity, bias=0.0,
                                                   scale=wcol)
                          if d_hi > L:
                              nc.vector.scalar_tensor_tensor(
                                  dst[:, L:d_hi], src[:, 0:d_hi - L], wcol,
                                  dst[:, L:d_hi], OP.mult, OP.add)
                          if tmp is not None:
                              nc.vector.tensor_tensor(
                                  dst[:, s_lo:s_hi], dst[:, s_lo:s_hi],
                                  tmp[:, 0:w], OP.add)

              if ph < 4:
                  nc.sync.dma_start(y_d[0:128, :], mxb_sb[:])
                  continue
              # ======== Phase 5: add memory output (pair 2), apply head weights ==
              with nc.named_scope("ph5_hw"):
               for ck in range(NCK):
                  cs = slice(512 * ck, 512 * (ck + 1))
                  nc.vector.tensor_tensor(C1[2][:, cs], C1[2][:, cs],
                                          mem_o[:, cs], OP.add)
               for p in range(3):
                  for ck in range(NCK):
                      cs = slice(512 * ck, 512 * (ck + 1))
                      ps_h = psp.tile([128, 512], f32, name="psh", tag="A")
                      nc.tensor.matmul(ps_h[:], eind_sb[:, 128 * p:128 * (p + 1)],
                                       hw_sb[:, cs], start=True, stop=True)
                      nc.vector.tensor_tensor(C1[p][:, cs], C1[p][:, cs],
                                              ps_h[:], OP.mult)

              if ph < 5:
                  nc.sync.dma_start(y_d[0:128, :], mxb_sb[:])
                  continue
              # ======== Phase 6: exchange (8-core AllToAll) ========
              with nc.named_scope("ph6_a2a"):
               bounce_in = dramp.tile([N_CORES * 384, TOK], f16, name="bin")
               bounce_out = dramp.tile([N_CORES * 384, TOK], f16, name="bout")
               for j in range(N_CORES):
                  for p in range(3):
                      nc.sync.dma_start(
                          bounce_in[384 * j + 128 * p:384 * j + 128 * (p + 1), :],
                          C1[p][:, TOK * j:TOK * (j + 1)])
               if sim:
                   # TimelineSim can't model collectives; equivalent-volume DMA
                   nc.sync.dma_start(bounce_out[:], bounce_in[:])
               else:
                   nc.gpsimd.collective_compute(
                      "AllToAll", mybir.AluOpType.bypass,
                      replica_groups=[list(range(N_CORES))],
                      ins=[bounce_in[:].opt()], outs=[bounce_out[:].opt()])
               hT = [mainp.tile([128, B * TOK], f16, name=f"ht{i}") for i in range(NB)]
               for fb in range(NB):
                  for b in range(B):
                      src_core = 2 * b + (1 if fb >= 3 else 0)
                      r0 = 384 * src_core + 128 * (fb % 3)
                      nc.sync.dma_start(hT[fb][:, TOK * b:TOK * (b + 1)],
                                        bounce_out[r0:r0 + 128, :])

              if ph < 6:
                  nc.sync.dma_start(y_d[0:128, :], mxb_sb[:])
                  continue
              # ======== Phase 7: mixing ========
              _s7 = nc.enter_named_scope("ph7_mix", False)[0]
              for tck in range(B * TOK // 512):
                  cs = slice(512 * tck, 512 * (tck + 1))
                  sigs = []
                  for fb in range(NB):
                      ps_pre = psp.tile([128, 512], f32, name="pre", tag="A")
                      for db in range(NB):
                          nc.tensor.matmul(ps_pre[:],
                                           mgT_sb[db][:, 128 * fb:128 * (fb + 1)],
                                           hT[db][:, cs], start=(db == 0),
                                           stop=(db == NB - 1))
                      sg = tmpp.tile([128, 512], f16, name=f"msig{fb}",
                                     tag=f"msig{fb}")
                      nc.scalar.activation(sg[:], ps_pre[:], AF.Sigmoid,
                                           bias=mgb_sb[:, fb:fb + 1], scale=1.0)
                      sigs.append(sg)
                  for fb in range(NB):
                      nc.vector.tensor_tensor(hT[fb][:, cs], hT[fb][:, cs],
                                              sigs[fb][:], OP.mult)
                  for tb in range(4):
                      tr = slice(512 * tck + 128 * tb, 512 * tck + 128 * (tb + 1))
                      for half in range(2):
                          ps_y = psp.tile([128, 384], f32, name="psy",
                                          tag=("B" if half == 0 else "C"))
                          for fb in range(NB):
                              nc.tensor.matmul(
                                  ps_y[:], hT[fb][:, tr],
                                  mxT_sb[fb][:, 384 * half:384 * (half + 1)],
                                  start=(fb == 0), stop=(fb == NB - 1))
                          y_sb = tmpp.tile([128, 384], f32, name="ysb",
                                           tag=f"ysb{half}")
                          nc.vector.tensor_tensor(
                              y_sb[:], ps_y[:],
                              mxb_sb[:, 384 * half:384 * (half + 1)], OP.add)
                          nc.sync.dma_start(
                              y_d[512 * tck + 128 * tb:512 * tck + 128 * (tb + 1),
                                  384 * half:384 * (half + 1)],
                              y_sb[:])
              nc.leave_named_scope("ph7_mix", _s7, False)

    nc.compile()
    return nc


def _prep_core_inputs(core, inp):
    b, g = core // 2, core % 2
    heads = GROUPS[g]
    f32, f16 = np.float32, np.float16

    x = np.asarray(inp["x"], f32)[b]
    gate_w = np.asarray(inp["gate_w"], f32)
    rows_xg = np.concatenate([np.arange(64 * h, 64 * h + 64) for h in heads])
    W_c = np.concatenate([gate_w[rows_xg], gate_w[768 + rows_xg]], axis=0)

    rT = np.zeros((HIDDEN, 8), f32)
    rT[:, :6] = np.asarray(inp["router_w"], f32)[heads].T
    rb = np.zeros((8, 1), f32)
    rb[:6, 0] = np.asarray(inp["router_b"], f32)[heads]

    conv_w = np.asarray(inp["conv_w"], f32)
    conv_b = np.asarray(inp["conv_b"], f32)
    csc = np.zeros((128, CONV_NCOL), f32)
    for p in range(3):
        for lay in range(3):
            sch = CONV_SCHED[p][lay]
            for hh in range(2):
                head = heads[2 * p + hh]
                rows = slice(64 * hh, 64 * (hh + 1))
                csc[rows, sch["scol"]] = 1.0 + conv_w[head, lay, :, 3]
                csc[rows, sch["bcol"]] = conv_b[head, lay, :]
                d = DILATIONS[head][lay]
                for (L, cw, *_rest) in sch["ops"]:
                    if L % d == 0 and 1 <= L // d <= 3:
                        csc[rows, cw] = conv_w[head, lay, :, 3 - L // d]

    ma, mb = heads[4], heads[5]
    ia, ib = MEM_HEADS.index(ma), MEM_HEADS.index(mb)
    Wq = np.asarray(inp["mem_Wq"], f32)
    Wk = np.asarray(inp["mem_Wk"], f32)
    Wv = np.asarray(inp["mem_Wv"], f32)
    Wgw = np.asarray(inp["mem_Wg_w"], f32)
    Wgb = np.asarray(inp["mem_Wg_b"], f32)
    Wo = np.asarray(inp["mem_Wout"], f32)

    qbd = np.zeros((128, 128), f32)
    qbd[0:64, 0:64] = Wq[ia].T
    qbd[64:128, 64:128] = Wq[ib].T
    kvg = np.zeros((128, 386), f32)
    kvg[0:64, 0:64] = Wk[ia].T
    kvg[64:128, 64:128] = Wk[ib].T
    kvg[0:64, 128:256] = Wv[ia].T
    kvg[64:128, 256:384] = Wv[ib].T
    kvg[0:64, 384] = Wgw[ia, 0]
    kvg[64:128, 385] = Wgw[ib, 0]
    gbb = np.zeros((128, 2), f32)
    gbb[:, 0] = Wgb[ia, 0]
    gbb[:, 1] = Wgb[ib, 0]
    wot = np.zeros((128, 256), f32)
    wot[:, 0:64] = Wo[ia].T           # head-a rows 0:64 of stacked out
    wot[:, 128 + 64:256] = Wo[ib].T   # head-b rows 64:128 of stacked out

    eind = np.zeros((8, 384), f32)
    for p in range(3):
        eind[2 * p, 128 * p:128 * p + 64] = 1.0
        eind[2 * p + 1, 128 * p + 64:128 * (p + 1)] = 1.0

    pf = np.concatenate([np.arange(64 * h, 64 * h + 64) for h in PERM_HEADS])
    mixg_w = np.asarray(inp["mixg_w"], f32)
    mix_w = np.asarray(inp["mix_w"], f32)

    return {
        "xT": np.ascontiguousarray(x.T).astype(f16),
        "wgT": np.ascontiguousarray(W_c.T).astype(f16),
        "rT": rT.astype(f16), "rb": rb, "conv_sc": csc,
        "mem_qbd": qbd.astype(f16), "mem_kvg": kvg.astype(f16),
        "mem_gb_bc": gbb, "mem_WoT": wot.astype(f16),
        "ones64": np.full((128, 64), 1.0 / 64.0, f32),
        "E_ind": eind.astype(f16),
        "mixgT": np.ascontiguousarray(mixg_w[np.ix_(pf, pf)].T).astype(f16),
        "mixgb": np.asarray(inp["mixg_b"], f32)[pf].reshape(HIDDEN, 1).copy(),
        "mixT": np.ascontiguousarray(mix_w[:, pf].T).astype(f16),
        "mixb_bc": np.tile(np.asarray(inp["mix_b"], f32)[None, :], (128, 1)),
    }


def prep_in_maps(inputs):
    return [_prep_core_inputs(c, inputs) for c in range(N_CORES)]


def get_bass():
    if "nc" not in _CACHE:
        _CACHE["nc"] = _build_bass()
    return _CACHE["nc"]


def assemble(results):
    out = np.zeros((B, S, HIDDEN), np.float32)
    for j in range(N_CORES):
        y = results[j]["y"].reshape(B, TOK, HIDDEN)
        out[:, TOK * j:TOK * (j + 1), :] = y
    return out


def kernel(**inputs):
    from concourse import bass_utils
    nc = get_bass()
    in_maps = prep_in_maps(inputs)
    res = bass_utils.run_bass_kernel_spmd(nc, in_maps,
                                          core_ids=list(range(N_CORES)))
    return assemble(res.results)



# revision 30
# speedup vs baseline: 95.8197x; 95.8197x over previous
"""Trainium2 Bass kernel for nn_MultiHeadDilatedState (B=4, S=4096, H=768).

Sharding: 8 cores = (batch b in 0..4) x (head-group g in 0..2); each core
runs the head phase (gate matmul + SwiGLU + dilated causal convs + neural
memory + router weighting) for its 6 heads over the full sequence in
feature-major layout, then two 8-core AllToAll waves re-shard
token-parallel: core j runs the mixing matmuls for token sub-windows
[2048w + 256j, 2048w + 256j + 256) (w = 0, 1) of every batch and outputs
token-major.  Host assembles the full output.

Conv emission: per (pair, layer), taps are lag-merged across both groups
and both head slots into full-128-partition ops whose per-core weight
column carries w (or 0 when inapplicable); head-to-pair assignment chosen
to maximize lag collisions.  Each layer's columns are split
DVE-stt | ScalarE-scale + DVE-add | Pool-scale+add so three engines work
concurrently without write overlap.  The conv runs in two column waves
(dst cols [0:2048), [2048:S)) so wave A overlaps the tail of phase 1 and
wave B overlaps exchange + mixing of wave A.  The neural-memory scan is
interleaved thunk-wise into wave A so its latency hides under conv work.

Self-contained: hardcodes all shapes; builds + compiles once per process.
"""
import numpy as np

DILATIONS = [(1, 2, 4), (1, 1, 1), (4, 8, 16), (8, 16, 32), (32, 64, 128),
             (64, 128, 256), (256, 512, 1024), (1, 100, 200), (1, 500, 1000),
             (1, 1024, 2048), (3, 9, 27), (5, 25, 125)]
MEM_HEADS = (6, 7, 8, 9)
HIDDEN = 768
B, S = 4, 4096
HS = S // 2          # conv column-wave boundary
N_CORES = 8
# pair quads chosen to maximize lag collisions (ops merge when k*d equal)
GROUPS = [[0, 1, 2, 3, 6, 8], [10, 11, 4, 5, 7, 9]]
PERM_HEADS = GROUPS[0] + GROUPS[1]
TOK = S // N_CORES   # 512
SUB = TOK // 2       # 256: per-wave sub-window
NB = HIDDEN // 128   # 6
NCK = S // 512       # 8

_CACHE = {}


def _conv_sched():
    """Lag-merged conv schedule with 3-engine column ownership.

    Returns (layers, ncol): layers[p][lay] = dict(scol, bcol, ops=[(lag,
    col, d_hi, s_lo, s_hi, p_lo)]): segment [lag:d_hi) DVE-stt,
    [s_lo:s_hi) ScalarE-scale + DVE-add, [p_lo:S) Pool scale+add.
    """
    quads = [[GROUPS[0][2 * p], GROUPS[0][2 * p + 1],
              GROUPS[1][2 * p], GROUPS[1][2 * p + 1]] for p in range(3)]
    lag_sets = []
    total_cols = 0
    for p in range(3):
        per_lay = []
        for lay in range(3):
            lags = sorted({k * DILATIONS[h][lay] for h in quads[p]
                           for k in (1, 2, 3) if k * DILATIONS[h][lay] < S})
            per_lay.append(lags)
            total_cols += sum(S - L for L in lags)
        lag_sets.append(per_lay)

    # us/kcol: route A = DVE stt 1.08; route C = ScalarE scale 0.93 + DVE
    # add 0.557; route F = Pool tscal+tt 3.42.  pre_* = non-conv engine
    # work sharing the conv window.  Bisect makespan T.
    tot = total_cols / 1000.0
    pre_d, pre_s, pool_slack = 44.0, 12.0, 15.0
    R_POOL = 3.42

    def feasible(T):
        b_p = max(T - pool_slack, 0.0) / R_POOL
        c_s = max((T - pre_s) / 0.93, 0.0)
        a_d = tot - b_p - c_s
        if a_d < 0:
            b_p = max(tot - c_s, 0.0)
            a_d = 0.0
        return 1.08 * a_d + 0.557 * c_s + pre_d <= T

    lo, hi = 10.0, 1000.0
    for _ in range(60):
        mid = (lo + hi) / 2
        if feasible(mid):
            hi = mid
        else:
            lo = mid
    T = hi
    b_p = min(max(T - pool_slack, 0.0) / R_POOL, tot)
    c_s = min(max((T - pre_s) / 0.93, 0.0), tot - b_p)
    f_pool = b_p / tot
    f_scal = c_s / tot

    def find_b(lags, target, from_hi):
        lo_, hi_ = 0, S
        for _ in range(40):
            m = (lo_ + hi_) // 2
            if from_hi:
                v = sum(S - max(L, m) for L in lags)
                if v > target:
                    lo_ = m + 1
                else:
                    hi_ = m
            else:
                v = sum(max(0, m - L) for L in lags)
                if v < target:
                    lo_ = m + 1
                else:
                    hi_ = m
        return (hi_ // 8) * 8

    layers = []
    col = 0
    for p in range(3):
        per_lay = []
        for lay in range(3):
            lags = lag_sets[p][lay]
            lcols = sum(S - L for L in lags)
            b1 = find_b(lags, f_pool * lcols, True)
            b0 = find_b(lags, (1.0 - f_pool - f_scal) * lcols, False)
            b0 = min(b0, b1)
            scol, bcol = col, col + 1
            col += 2
            ops = []
            for L in lags:
                d_hi = max(L, b0)
                s_lo, s_hi = max(L, b0), max(L, b1)
                p_lo = max(L, b1)
                if S - p_lo < 384:  # pool seg too small: fold into scal seg
                    s_hi = S
                    p_lo = S
                ops.append((L, col, d_hi, s_lo, s_hi, p_lo))
                col += 1
            per_lay.append(dict(scol=scol, bcol=bcol, ops=ops))
        layers.append(per_lay)
    return layers, col


CONV_SCHED, CONV_NCOL = _conv_sched()


def _build_bass(reps=1, sim=False):
    import concourse.bacc as bacc
    import concourse.mybir as mybir
    import concourse.tile as tile

    f32 = mybir.dt.float32
    f16 = mybir.dt.float16
    AF = mybir.ActivationFunctionType
    OP = mybir.AluOpType

    nc = bacc.Bacc("TRN2", target_bir_lowering=False, debug=False,
                   num_devices=N_CORES)

    def din(name, shape, dt=f32):
        return nc.dram_tensor(name, shape, dt, kind="ExternalInput").ap()

    xT_d = din("xT", [HIDDEN, S], f16)
    wgT_d = din("wgT", [HIDDEN, HIDDEN], f16)
    rT_d = din("rT", [HIDDEN, 8], f16)
    rb_d = din("rb", [8, 1])
    csc_d = din("conv_sc", [128, CONV_NCOL])
    qbd_d = din("mem_qbd", [128, 128], f16)
    kvg_d = din("mem_kvg", [128, 386], f16)
    gbb_d = din("mem_gb_bc", [128, 2])
    wot_d = din("mem_WoT", [128, 256], f16)
    ones_d = din("ones64", [128, 64])
    eind_d = din("E_ind", [8, 384], f16)
    mgT_d = din("mixgT", [HIDDEN, HIDDEN], f16)
    mgb_d = din("mixgb", [HIDDEN, 1])
    mxT_d = din("mixT", [HIDDEN, HIDDEN], f16)
    mxb_d = din("mixb_bc", [128, HIDDEN])
    y_d = nc.dram_tensor("y", [B * TOK, HIDDEN], f16, kind="ExternalOutput").ap()

    with tile.TileContext(nc) as tc:
        with (
            tc.tile_pool(name="const", bufs=1) as constp,
            tc.tile_pool(name="main", bufs=1) as mainp,
            tc.tile_pool(name="xt", bufs=2) as xtp,
            tc.tile_pool(name="tmp", bufs=3) as tmpp,
            tc.tile_pool(name="ps", bufs=2, space="PSUM") as psp,
            tc.tile_pool(name="dram", bufs=1, space="DRAM") as dramp,
        ):
            # ---- resident weights: phase-1-critical on sync queue, the
            # rest on the Activation DGE queue so x streaming is unblocked
            wg_sb = [constp.tile([128, HIDDEN], f16, name=f"wg{i}") for i in range(NB)]
            rT_sb = [constp.tile([128, 8], f16, name=f"rt{i}") for i in range(NB)]
            for i in range(NB):
                nc.sync.dma_start(wg_sb[i][:], wgT_d[128 * i:128 * (i + 1), :])
                nc.sync.dma_start(rT_sb[i][:], rT_d[128 * i:128 * (i + 1), :])
            rb_sb = constp.tile([8, 1], f32, name="rb")
            nc.sync.dma_start(rb_sb[:], rb_d[:])
            csc_sb = constp.tile([128, CONV_NCOL], f32, name="csc")
            nc.scalar.dma_start(csc_sb[:], csc_d[:])
            qbd_sb = constp.tile([128, 128], f16, name="qbd")
            nc.scalar.dma_start(qbd_sb[:], qbd_d[:])
            kvg_sb = constp.tile([128, 386], f16, name="kvgw")
            nc.scalar.dma_start(kvg_sb[:], kvg_d[:])
            gbb_sb = constp.tile([128, 2], f32, name="gbb")
            nc.scalar.dma_start(gbb_sb[:], gbb_d[:])
            wot_sb = constp.tile([128, 256], f16, name="wot")
            nc.scalar.dma_start(wot_sb[:], wot_d[:])
            ones_sb = constp.tile([128, 64], f32, name="ones")
            nc.scalar.dma_start(ones_sb[:], ones_d[:])
            eind_sb = constp.tile([8, 384], f16, name="eind")
            nc.scalar.dma_start(eind_sb[:], eind_d[:])
            mgT_sb = [constp.tile([128, HIDDEN], f16, name=f"mg{i}") for i in range(NB)]
            mxT_sb = [constp.tile([128, HIDDEN], f16, name=f"mx{i}") for i in range(NB)]
            for i in range(NB):
                nc.scalar.dma_start(mgT_sb[i][:], mgT_d[128 * i:128 * (i + 1), :])
                nc.scalar.dma_start(mxT_sb[i][:], mxT_d[128 * i:128 * (i + 1), :])
            mgb_sb = constp.tile([128, NB], f32, name="mgb")
            for i in range(NB):
                nc.scalar.dma_start(mgb_sb[:, i:i + 1], mgb_d[128 * i:128 * (i + 1), :])
            mxb_sb = constp.tile([128, HIDDEN], f32, name="mxb")
            nc.scalar.dma_start(mxb_sb[:], mxb_d[:])

            max_sw = max((o[4] - o[3] for pl in CONV_SCHED for sch in pl
                          for o in sch["ops"]), default=0)
            max_pw = max((S - o[5] for pl in CONV_SCHED for sch in pl
                          for o in sch["ops"]), default=0)

            for _rep in range(reps):
              xg = [mainp.tile([128, S], f16, name=f"xg{p}", tag=f"xg{p}") for p in range(3)]
              C1 = [mainp.tile([128, S], f16, name=f"c1_{p}", tag=f"c1_{p}") for p in range(3)]
              C2m = mainp.tile([128, S], f16, name="c2m", tag="c2m")
              hw_sb = mainp.tile([8, S], f16, name="hww", tag="hww")

              # ======== Phase 1: gate matmul + SwiGLU + router ========
              with nc.named_scope("ph1_gate"):
               for ck in range(NCK):
                  cs = slice(512 * ck, 512 * (ck + 1))
                  xt = [xtp.tile([128, 512], f16, name=f"xt{i}", tag=f"xt{i}")
                        for i in range(NB)]
                  for i in range(NB):
                      nc.sync.dma_start(xt[i][:], xT_d[128 * i:128 * (i + 1), cs])
                  ps_r = psp.tile([8, 512], f32, name="psr", tag="C", bufs=1)
                  for db in range(NB):
                      nc.tensor.matmul(ps_r[:], rT_sb[db][:], xt[db][:],
                                       start=(db == 0), stop=(db == NB - 1))
                  nc.scalar.activation(hw_sb[:, cs], ps_r[:], AF.Sigmoid,
                                       bias=rb_sb[:, 0:1], scale=1.0)
                  for pb in range(3):
                      ps_a = psp.tile([128, 512], f32, name="psa", tag="A")
                      ps_b = psp.tile([128, 512], f32, name="psb", tag="B")
                      for db in range(NB):
                          nc.tensor.matmul(
                              ps_a[:], wg_sb[db][:, 128 * pb:128 * (pb + 1)],
                              xt[db][:], start=(db == 0), stop=(db == NB - 1))
                      for db in range(NB):
                          nc.tensor.matmul(
                              ps_b[:],
                              wg_sb[db][:, 384 + 128 * pb:384 + 128 * (pb + 1)],
                              xt[db][:], start=(db == 0), stop=(db == NB - 1))
                      sig = tmpp.tile([128, 512], f32, name="sig", tag="sig")
                      nc.scalar.activation(sig[:], ps_b[:], AF.Sigmoid)
                      nc.vector.tensor_tensor(xg[pb][:, cs], ps_a[:], sig[:], OP.mult)

              # ======== neural memory scan (thunks; interleaved below) ====
              x_mem = xg[2]
              rd_ck = [mainp.tile([128, 512], f16, name=f"rdck{h}", tag=f"rdck{h}") for h in range(2)]
              mem_o = mainp.tile([128, S], f16, name="memo", tag="memo")
              M_a = mainp.tile([64, 128], f32, name="Ma", tag="Ma")
              M_b = mainp.tile([64, 128], f32, name="Mb", tag="Mb")
              nc.vector.memset(M_a[:], 0.0)
              nc.vector.memset(M_b[:], 0.0)

              def scan_thunks():
                  for ck4 in range(NCK):
                      # q projection for a full 512-chunk, both heads
                      cs4 = slice(512 * ck4, 512 * (ck4 + 1))
                      ps_qa = psp.tile([64, 512], f32, name="psqa", tag="D", bufs=1)
                      ps_qb = psp.tile([64, 512], f32, name="psqb", tag="E", bufs=1)
                      q_a = tmpp.tile([64, 512], f32, name="qa", tag="qa", bufs=2)
                      q_b = tmpp.tile([64, 512], f32, name="qb", tag="qb", bufs=2)

                      def q_proj(cs4=cs4, ps_qa=ps_qa, ps_qb=ps_qb, q_a=q_a, q_b=q_b):
                          nc.tensor.matmul(ps_qa[:], qbd_sb[:, 0:64],
                                           x_mem[:, cs4], start=True, stop=True)
                          nc.tensor.matmul(ps_qb[:], qbd_sb[:, 64:128],
                                           x_mem[:, cs4], start=True, stop=True)
                          nc.scalar.copy(q_a[:], ps_qa[:])
                          nc.scalar.copy(q_b[:], ps_qb[:])
                      yield q_proj
                      for bi in range(4):
                          blk = 4 * ck4 + bi
                          bs = slice(128 * blk, 128 * (blk + 1))
                          ps_rd = psp.tile([128, 256], f32, name="psrd", tag="F", bufs=1)
                          for half in range(2):
                              def step(blk=blk, half=half, bi=bi, ps_rd=ps_rd,
                                       q_a=q_a, q_b=q_b):
                                  c64 = slice(128 * blk + 64 * half,
                                              128 * blk + 64 * (half + 1))
                                  qc = slice(128 * bi + 64 * half,
                                             128 * bi + 64 * (half + 1))
                                  ps_kvg = psp.tile([64, 386], f32, name="pskvg",
                                                    tag="C", bufs=1)
                                  nc.tensor.matmul(ps_kvg[:], x_mem[:, c64],
                                                   kvg_sb[:], start=True, stop=True)
                                  g_sb = tmpp.tile([64, 2], f32, name="gsb", tag="gsb")
                                  for hh in range(2):
                                      nc.scalar.activation(
                                          g_sb[:, hh:hh + 1],
                                          ps_kvg[:, 384 + hh:385 + hh], AF.Sigmoid,
                                          bias=gbb_sb[0:64, hh:hh + 1], scale=1.0)
                                  kg_sb = tmpp.tile([64, 128], f16, name="kgsb", tag="kgsb")
                                  for hh in range(2):
                                      nc.vector.tensor_scalar(
                                          kg_sb[:, 64 * hh:64 * (hh + 1)],
                                          ps_kvg[:, 64 * hh:64 * (hh + 1)],
                                          g_sb[:, hh:hh + 1], None, OP.mult)
                                  v_sb = tmpp.tile([64, 256], f16, name="vsb", tag="vsb")
                                  nc.scalar.copy(v_sb[:], ps_kvg[:, 128:384])
                                  nc.tensor.matmul(
                                      ps_rd[:, 64 * half:64 * (half + 1)],
                                      M_a[:], q_a[:, qc], start=True, stop=True)
                                  nc.tensor.matmul(
                                      ps_rd[:, 128 + 64 * half:128 + 64 * (half + 1)],
                                      M_b[:], q_b[:, qc], start=True, stop=True)
                                  ps_g = psp.tile([64, 2], f32, name="psg", tag="E", bufs=1)
                                  nc.tensor.matmul(ps_g[:], ones_sb[0:64, :], g_sb[:],
                                                   start=True, stop=True)
                                  decay = tmpp.tile([64, 2], f32, name="decay", tag="decay")
                                  nc.scalar.activation(decay[:], ps_g[:], AF.Identity,
                                                       bias=1.0, scale=-1.0)
                                  ps_w = psp.tile([64, 256], f32, name="psw", tag="D", bufs=1)
                                  nc.tensor.matmul(ps_w[:, 0:128], kg_sb[:, 0:64],
                                                   v_sb[:, 0:128], start=True, stop=True)
                                  nc.tensor.matmul(ps_w[:, 128:256], kg_sb[:, 64:128],
                                                   v_sb[:, 128:256], start=True, stop=True)
                                  nc.vector.scalar_tensor_tensor(
                                      M_a[:], M_a[:], decay[:, 0:1], ps_w[:, 0:128],
                                      OP.mult, OP.add)
                                  nc.vector.scalar_tensor_tensor(
                                      M_b[:], M_b[:], decay[:, 1:2], ps_w[:, 128:256],
                                      OP.mult, OP.add)
                              yield step

                          def evict(blk=blk, bi=bi, ps_rd=ps_rd):
                              cc = 128 * bi
                              for hh in range(2):
                                  nc.scalar.copy(rd_ck[hh][:, cc:cc + 128],
                                                 ps_rd[:, 128 * hh:128 * (hh + 1)])
                          yield evict
                      def wout(ck4=ck4, cs4=cs4):
                          ps_o = psp.tile([128, 512], f32, name="pso", tag="C", bufs=1)
                          nc.tensor.matmul(ps_o[:], wot_sb[:, 0:128], rd_ck[0][:],
                                           start=True, stop=False)
                          nc.tensor.matmul(ps_o[:], wot_sb[:, 128:256], rd_ck[1][:],
                                           start=False, stop=True)
                          nc.scalar.copy(mem_o[:, cs4], ps_o[:])
                      yield wout

              # ======== conv thunks (two column waves) ========
              def chain_tiles(p):
                  if p < 2:
                      return [(xg[p], C1[p]), (C1[p], xg[p]), (xg[p], C1[p])]
                  return [(xg[2], C1[2]), (C1[2], C2m), (C2m, C1[2])]

              ctrs = {"ct": 0, "pt": 0}

              # wave-B of layer l reads layer-(l-1) output at cols < HS that
              # wave-A of the clobbering layer overwrites (ping-pong tile
              # reuse).  Snapshot the max-lag-wide boundary region before the
              # clobber; wave-B reads split at HS between snapshot and live.
              def laymax(p, lay):
                  return min(max((o[0] for o in CONV_SCHED[p][lay]["ops"]),
                                 default=0), HS)
              # snap_x[p]: original xg[p] (clobbered by lay1-A dst, p<2)
              # snap_0[p]: lay0 output (clobbered by lay2-A dst)
              snW_x = [laymax(p, 0) if p < 2 else 0 for p in range(3)]
              snW_0 = [laymax(p, 1) for p in range(3)]
              snap_x = [mainp.tile([128, max(snW_x[p], 8)], f16,
                                   name=f"snx{p}", tag=f"snx{p}")
                        for p in range(3)]
              snap_0 = [mainp.tile([128, max(snW_0[p], 8)], f16,
                                   name=f"sn0{p}", tag=f"sn0{p}")
                        for p in range(3)]

              def conv_thunks(wave):
                  w_lo, w_hi = (0, HS) if wave == 0 else (HS, S)
                  for lay in range(3):
                      for p in range(3):
                          src, dst = chain_tiles(p)[lay]
                          sch = CONV_SCHED[p][lay]
                          # wave-A: snapshot what this layer's dst clobbers
                          snap = None
                          if wave == 0:
                              if lay == 1 and p < 2 and snW_x[p] > 0:
                                  snap = (snap_x[p], snW_x[p], dst)
                              elif lay == 2 and snW_0[p] > 0:
                                  # lay2 dst holds lay0 output (= lay1 src)
                                  snap = (snap_0[p], snW_0[p], dst)
                          # wave-B: which snapshot replaces sub-HS src reads
                          rd_snap = None
                          if wave == 1:
                              if lay == 0 and p < 2 and snW_x[p] > 0:
                                  rd_snap = (snap_x[p], snW_x[p])
                              elif lay == 1 and snW_0[p] > 0:
                                  rd_snap = (snap_0[p], snW_0[p])

                          def base(src=src, dst=dst, sch=sch, snap=snap):
                              if snap is not None:
                                  st, sw, stile = snap
                                  nc.vector.tensor_copy(st[:, 0:sw],
                                                        stile[:, HS - sw:HS])
                              nc.vector.tensor_scalar(
                                  dst[:, w_lo:w_hi], src[:, w_lo:w_hi],
                                  csc_sb[:, sch["scol"]:sch["scol"] + 1],
                                  csc_sb[:, sch["bcol"]:sch["bcol"] + 1],
                                  OP.mult, OP.add)
                          yield base

                          def seg_pieces(lo, hi, L, rd_snap):
                              # split [lo,hi) dst cols at src boundary HS
                              if hi <= lo:
                                  return
                              if rd_snap is None or lo - L >= HS:
                                  yield (None, lo, hi)
                                  return
                              cut = min(hi, HS + L)
                              yield (rd_snap, lo, cut)
                              if hi > cut:
                                  yield (None, cut, hi)

                          for (L, cw, d_hi, s_lo, s_hi, p_lo) in sch["ops"]:
                              dl, dh = max(L, w_lo), min(d_hi, w_hi)
                              sl, sh = max(s_lo, w_lo), min(s_hi, w_hi)
                              pl, ph_ = max(p_lo, w_lo), w_hi
                              wcol = csc_sb[:, cw:cw + 1]

                              def tap(src=src, dst=dst, wcol=wcol, L=L, dl=dl,
                                      dh=dh, sl=sl, sh=sh, pl=pl, ph_=ph_,
                                      rd_snap=rd_snap):
                                  def sview(piece, lo, hi):
                                      rs, _, _ = piece
                                      if rs is None:
                                          return src[:, lo - L:hi - L]
                                      st, sw = rs
                                      off = HS - sw
                                      return st[:, lo - L - off:hi - L - off]
                                  for piece in seg_pieces(pl, ph_, L, rd_snap):
                                      _, lo, hi = piece
                                      pw = hi - lo
                                      ptmp = tmpp.tile([128, max_pw], f16,
                                                       name="pt",
                                                       tag=f"pt{ctrs['pt'] % 2}",
                                                       bufs=1)
                                      ctrs['pt'] += 1
                                      nc.gpsimd.tensor_scalar(
                                          ptmp[:, 0:pw], sview(piece, lo, hi),
                                          wcol, None, OP.mult)
                                      nc.gpsimd.tensor_tensor(
                                          dst[:, lo:hi], dst[:, lo:hi],
                                          ptmp[:, 0:pw], OP.add)
                                  for piece in seg_pieces(sl, sh, L, rd_snap):
                                      _, lo, hi = piece
                                      w = hi - lo
                                      tmp = tmpp.tile([128, max_sw], f16,
                                                      name="ct",
                                                      tag=f"ct{ctrs['ct'] % 3}",
                                                      bufs=1)
                                      ctrs['ct'] += 1
                                      nc.scalar.activation(
                                          tmp[:, 0:w], sview(piece, lo, hi),
                                          AF.Identity, bias=0.0, scale=wcol)
                                      nc.vector.tensor_tensor(
                                          dst[:, lo:hi], dst[:, lo:hi],
                                          tmp[:, 0:w], OP.add)
                                  for piece in seg_pieces(dl, dh, L, rd_snap):
                                      _, lo, hi = piece
                                      nc.vector.scalar_tensor_tensor(
                                          dst[:, lo:hi], sview(piece, lo, hi),
                                          wcol, dst[:, lo:hi], OP.mult, OP.add)
                              yield tap

              # ======== ph5: memory add + head weights (per wave) ========
              def ph5(wave):
                  for ck in range(4 * wave, 4 * wave + 4):
                      cs = slice(512 * ck, 512 * (ck + 1))
                      nc.vector.tensor_tensor(C1[2][:, cs], C1[2][:, cs],
                                              mem_o[:, cs], OP.add)
                  for p in range(3):
                      for ck in range(4 * wave, 4 * wave + 4):
                          cs = slice(512 * ck, 512 * (ck + 1))
                          ps_h = psp.tile([128, 512], f32, name="psh", tag="A")
                          nc.tensor.matmul(ps_h[:], eind_sb[:, 128 * p:128 * (p + 1)],
                                           hw_sb[:, cs], start=True, stop=True)
                          nc.vector.tensor_tensor(C1[p][:, cs], C1[p][:, cs],
                                                  ps_h[:], OP.mult)

              # ======== a2a wave: bounce out, collective, gather hT ========
              hT = [[mainp.tile([128, B * SUB], f16, name=f"ht{w}_{i}",
                               tag=f"ht{w}_{i}") for i in range(NB)]
                    for w in range(2)]

              def a2a(wave):
                  b_in = dramp.tile([N_CORES * 384, SUB], f16, name=f"bin{wave}",
                                    tag=f"bin{wave}")
                  b_out = dramp.tile([N_CORES * 384, SUB], f16, name=f"bout{wave}",
                                     tag=f"bout{wave}")
                  for j in range(N_CORES):
                      ts = slice(HS * wave + SUB * j, HS * wave + SUB * (j + 1))
                      for p in range(3):
                          nc.sync.dma_start(
                              b_in[384 * j + 128 * p:384 * j + 128 * (p + 1), :],
                              C1[p][:, ts])
                  if sim:
                      nc.sync.dma_start(b_out[:], b_in[:])
                  else:
                      import concourse.mybir as mybir
                      nc.gpsimd.collective_compute(
                          "AllToAll", mybir.AluOpType.bypass,
                          replica_groups=[list(range(N_CORES))],
                          ins=[b_in[:].opt()], outs=[b_out[:].opt()])
                  for fb in range(NB):
                      for b in range(B):
                          src_core = 2 * b + (1 if fb >= 3 else 0)
                          r0 = 384 * src_core + 128 * (fb % 3)
                          nc.sync.dma_start(hT[wave][fb][:, SUB * b:SUB * (b + 1)],
                                            b_out[r0:r0 + 128, :])

              # ======== ph7: mixing for one wave's sub-windows ========
              def ph7_thunks(wave):
                  ht = hT[wave]
                  for b in range(B):
                    def mixg_b(b=b):
                      cs = slice(SUB * b, SUB * (b + 1))
                      sigs = []
                      for fb in range(NB):
                          ps_pre = psp.tile([128, 256], f32, name="pre", tag="A")
                          for db in range(NB):
                              nc.tensor.matmul(ps_pre[:],
                                               mgT_sb[db][:, 128 * fb:128 * (fb + 1)],
                                               ht[db][:, cs], start=(db == 0),
                                               stop=(db == NB - 1))
                          sg = tmpp.tile([128, 256], f16, name=f"msig{fb}",
                                         tag=f"msig{fb}")
                          nc.scalar.activation(sg[:], ps_pre[:], AF.Sigmoid,
                                               bias=mgb_sb[:, fb:fb + 1], scale=1.0)
                          sigs.append(sg)
                      for fb in range(NB):
                          nc.vector.tensor_tensor(ht[fb][:, cs], ht[fb][:, cs],
                                                  sigs[fb][:], OP.mult)
                    yield mixg_b
                    def mix_b(b=b):
                      cs = slice(SUB * b, SUB * (b + 1))
                      for tb in range(2):
                          tr = slice(SUB * b + 128 * tb, SUB * b + 128 * (tb + 1))
                          yr = 512 * b + 256 * wave + 128 * tb
                          for half in range(2):
                              ps_y = psp.tile([128, 384], f32, name="psy",
                                              tag=("B" if half == 0 else "C"),
                                              bufs=(2 if half == 0 else 1))
                              for fb in range(NB):
                                  nc.tensor.matmul(
                                      ps_y[:], ht[fb][:, tr],
                                      mxT_sb[fb][:, 384 * half:384 * (half + 1)],
                                      start=(fb == 0), stop=(fb == NB - 1))
                              y_sb = tmpp.tile([128, 384], f16, name="ysb",
                                               tag=f"ysb{half}")
                              nc.vector.tensor_tensor(
                                  y_sb[:], ps_y[:],
                                  mxb_sb[:, 384 * half:384 * (half + 1)], OP.add)
                              nc.sync.dma_start(
                                  y_d[yr:yr + 128,
                                      384 * half:384 * (half + 1)],
                                  y_sb[:])
                    yield mix_b

              # ======== drive: interleave scan into conv wave A ========
              with nc.named_scope("ph3_convA"):
                  convA = list(conv_thunks(0))
                  scan = list(scan_thunks())
                  front = 15
                  for t in convA[:front]:
                      t()
                  rest = convA[front:]
                  si, n_s, n_c = 0, len(scan), len(rest)
                  for i, t in enumerate(rest):
                      t()
                      want = (i + 1) * n_s // max(n_c, 1)
                      while si < min(want, n_s):
                          scan[si]()
                          si += 1
                  while si < n_s:
                      scan[si]()
                      si += 1
              with nc.named_scope("ph5_A"):
                  ph5(0)
              with nc.named_scope("a2a_A"):
                  a2a(0)
              with nc.named_scope("ph3_convB"):
                  convB = list(conv_thunks(1))
                  mixA = list(ph7_thunks(0))
                  frontB = max(len(convB) // 3, 1)
                  for t in convB[:frontB]:
                      t()
                  restB = convB[frontB:]
                  mi, n_m, n_cb = 0, len(mixA), len(restB)
                  for i, t in enumerate(restB):
                      t()
                      want = (i + 1) * n_m // max(n_cb, 1)
                      while mi < min(want, n_m):
                          mixA[mi]()
                          mi += 1
                  while mi < n_m:
                      mixA[mi]()
                      mi += 1
              with nc.named_scope("ph5_B"):
                  ph5(1)
              with nc.named_scope("a2a_B"):
                  a2a(1)
              with nc.named_scope("ph7_B"):
                  for t in ph7_thunks(1):
                      t()

    nc.compile()
    return nc


def _prep_core_inputs(core, inp):
    b, g = core // 2, core % 2
    heads = GROUPS[g]
    f32, f16 = np.float32, np.float16

    x = np.asarray(inp["x"], f32)[b]
    gate_w = np.asarray(inp["gate_w"], f32)
    rows_xg = np.concatenate([np.arange(64 * h, 64 * h + 64) for h in heads])
    W_c = np.concatenate([gate_w[rows_xg], gate_w[768 + rows_xg]], axis=0)

    rT = np.zeros((HIDDEN, 8), f32)
    rT[:, :6] = np.asarray(inp["router_w"], f32)[heads].T
    rb = np.zeros((8, 1), f32)
    rb[:6, 0] = np.asarray(inp["router_b"], f32)[heads]

    conv_w = np.asarray(inp["conv_w"], f32)
    conv_b = np.asarray(inp["conv_b"], f32)
    csc = np.zeros((128, CONV_NCOL), f32)
    for p in range(3):
        for lay in range(3):
            sch = CONV_SCHED[p][lay]
            for hh in range(2):
                head = heads[2 * p + hh]
                rows = slice(64 * hh, 64 * (hh + 1))
                csc[rows, sch["scol"]] = 1.0 + conv_w[head, lay, :, 3]
                csc[rows, sch["bcol"]] = conv_b[head, lay, :]
                d = DILATIONS[head][lay]
                for (L, cw, *_rest) in sch["ops"]:
                    if L % d == 0 and 1 <= L // d <= 3:
                        csc[rows, cw] = conv_w[head, lay, :, 3 - L // d]

    ma, mb = heads[4], heads[5]
    ia, ib = MEM_HEADS.index(ma), MEM_HEADS.index(mb)
    Wq = np.asarray(inp["mem_Wq"], f32)
    Wk = np.asarray(inp["mem_Wk"], f32)
    Wv = np.asarray(inp["mem_Wv"], f32)
    Wgw = np.asarray(inp["mem_Wg_w"], f32)
    Wgb = np.asarray(inp["mem_Wg_b"], f32)
    Wo = np.asarray(inp["mem_Wout"], f32)

    qbd = np.zeros((128, 128), f32)
    qbd[0:64, 0:64] = Wq[ia].T
    qbd[64:128, 64:128] = Wq[ib].T
    kvg = np.zeros((128, 386), f32)
    kvg[0:64, 0:64] = Wk[ia].T
    kvg[64:128, 64:128] = Wk[ib].T
    kvg[0:64, 128:256] = Wv[ia].T
    kvg[64:128, 256:384] = Wv[ib].T
    kvg[0:64, 384] = Wgw[ia, 0]
    kvg[64:128, 385] = Wgw[ib, 0]
    gbb = np.zeros((128, 2), f32)
    gbb[:, 0] = Wgb[ia, 0]
    gbb[:, 1] = Wgb[ib, 0]
    wot = np.zeros((128, 256), f32)
    wot[:, 0:64] = Wo[ia].T           # head-a rows 0:64 of stacked out
    wot[:, 128 + 64:256] = Wo[ib].T   # head-b rows 64:128 of stacked out

    eind = np.zeros((8, 384), f32)
    for p in range(3):
        eind[2 * p, 128 * p:128 * p + 64] = 1.0
        eind[2 * p + 1, 128 * p + 64:128 * (p + 1)] = 1.0

    pf = np.concatenate([np.arange(64 * h, 64 * h + 64) for h in PERM_HEADS])
    mixg_w = np.asarray(inp["mixg_w"], f32)
    mix_w = np.asarray(inp["mix_w"], f32)

    return {
        "xT": np.ascontiguousarray(x.T).astype(f16),
        "wgT": np.ascontiguousarray(W_c.T).astype(f16),
        "rT": rT.astype(f16), "rb": rb, "conv_sc": csc,
        "mem_qbd": qbd.astype(f16), "mem_kvg": kvg.astype(f16),
        "mem_gb_bc": gbb, "mem_WoT": wot.astype(f16),
        "ones64": np.full((128, 64), 1.0 / 64.0, f32),
        "E_ind": eind.astype(f16),
        "mixgT": np.ascontiguousarray(mixg_w[np.ix_(pf, pf)].T).astype(f16),
        "mixgb": np.asarray(inp["mixg_b"], f32)[pf].reshape(HIDDEN, 1).copy(),
        "mixT": np.ascontiguousarray(mix_w[:, pf].T).astype(f16),
        "mixb_bc": np.tile(np.asarray(inp["mix_b"], f32)[None, :], (128, 1)),
    }


def prep_in_maps(inputs):
    return [_prep_core_inputs(c, inputs) for c in range(N_CORES)]


def get_bass():
    if "nc" not in _CACHE:
        _CACHE["nc"] = _build_bass()
    return _CACHE["nc"]


def assemble(results):
    out = np.zeros((B, S, HIDDEN), np.float32)
    for j in range(N_CORES):
        y = results[j]["y"].astype(np.float32)   # [B*TOK, H] f16
        for b in range(B):
            for w in range(2):
                rows = y[512 * b + 256 * w:512 * b + 256 * w + 256]
                out[b, HS * w + SUB * j:HS * w + SUB * (j + 1), :] = rows
    return out


def kernel(**inputs):
    from concourse import bass_utils
    nc = get_bass()
    in_maps = prep_in_maps(inputs)
    res = bass_utils.run_bass_kernel_spmd(nc, in_maps,
                                          core_ids=list(range(N_CORES)))
    return assemble(res.results)


# revision 32
# speedup vs baseline: 115.8274x; 1.2088x over previous
"""Trainium2 Bass kernel for nn_MultiHeadDilatedState (B=4, S=4096, H=768).

Sharding: 8 cores = (batch b in 0..4) x (head-group g in 0..2); each core
runs the head phase (gate matmul + SwiGLU + dilated causal convs + neural
memory + router weighting) for its 6 heads over the full sequence in
feature-major layout, then two 8-core AllToAll waves re-shard
token-parallel: core j runs the mixing matmuls for token sub-windows
[2048w + 256j, 2048w + 256j + 256) (w = 0, 1) of every batch and outputs
token-major.  Host assembles the full output.

Conv emission: per (pair, layer), taps are lag-merged across both groups
and both head slots into full-128-partition ops whose per-core weight
column carries w (or 0 when inapplicable); head-to-pair assignment chosen
to maximize lag collisions.  Each layer's columns are split
DVE-stt | ScalarE-scale + DVE-add | Pool-scale+add so three engines work
concurrently without write overlap.  The conv runs in two column waves
(dst cols [0:2048), [2048:S)) so wave A overlaps the tail of phase 1 and
wave B overlaps exchange + mixing of wave A.  The neural-memory scan is
interleaved thunk-wise into wave A so its latency hides under conv work.

Self-contained: hardcodes all shapes; builds + compiles once per process.
"""
import numpy as np

DILATIONS = [(1, 2, 4), (1, 1, 1), (4, 8, 16), (8, 16, 32), (32, 64, 128),
             (64, 128, 256), (256, 512, 1024), (1, 100, 200), (1, 500, 1000),
             (1, 1024, 2048), (3, 9, 27), (5, 25, 125)]
MEM_HEADS = (6, 7, 8, 9)
HIDDEN = 768
B, S = 4, 4096
HS = S // 2          # conv column-wave boundary
N_CORES = 8
# pair quads chosen to maximize lag collisions (ops merge when k*d equal)
GROUPS = [[0, 1, 2, 3, 6, 8], [10, 11, 4, 5, 7, 9]]
PERM_HEADS = GROUPS[0] + GROUPS[1]
TOK = S // N_CORES   # 512
SUB = TOK // 2       # 256: per-wave sub-window
NB = HIDDEN // 128   # 6
NCK = S // 512       # 8

_CACHE = {}


def _conv_sched():
    """Lag-merged conv schedule with 3-engine column ownership.

    Returns (layers, ncol): layers[p][lay] = dict(scol, bcol, ops=[(lag,
    col, d_hi, s_lo, s_hi, p_lo)]): segment [lag:d_hi) DVE-stt,
    [s_lo:s_hi) ScalarE-scale + DVE-add, [p_lo:S) Pool scale+add.
    """
    quads = [[GROUPS[0][2 * p], GROUPS[0][2 * p + 1],
              GROUPS[1][2 * p], GROUPS[1][2 * p + 1]] for p in range(3)]
    lag_sets = []
    total_cols = 0
    for p in range(3):
        per_lay = []
        for lay in range(3):
            lags = sorted({k * DILATIONS[h][lay] for h in quads[p]
                           for k in (1, 2, 3) if k * DILATIONS[h][lay] < S})
            per_lay.append(lags)
            total_cols += sum(S - L for L in lags)
        lag_sets.append(per_lay)

    # us/kcol: route A = DVE stt 1.08; route C = ScalarE scale 0.93 + DVE
    # add 0.557; route F = Pool tscal+tt 3.42.  pre_* = non-conv engine
    # work sharing the conv window.  Bisect makespan T.
    tot = total_cols / 1000.0
    pre_d, pre_s, pool_slack = 44.0, 12.0, 15.0
    R_POOL = 3.42

    def feasible(T):
        b_p = max(T - pool_slack, 0.0) / R_POOL
        c_s = max((T - pre_s) / 0.93, 0.0)
        a_d = tot - b_p - c_s
        if a_d < 0:
            b_p = max(tot - c_s, 0.0)
            a_d = 0.0
        return 1.08 * a_d + 0.557 * c_s + pre_d <= T

    lo, hi = 10.0, 1000.0
    for _ in range(60):
        mid = (lo + hi) / 2
        if feasible(mid):
            hi = mid
        else:
            lo = mid
    T = hi
    b_p = min(max(T - pool_slack, 0.0) / R_POOL, tot)
    c_s = min(max((T - pre_s) / 0.93, 0.0), tot - b_p)
    f_pool = b_p / tot
    f_scal = c_s / tot

    def find_b(lags, target, from_hi):
        lo_, hi_ = 0, S
        for _ in range(40):
            m = (lo_ + hi_) // 2
            if from_hi:
                v = sum(S - max(L, m) for L in lags)
                if v > target:
                    lo_ = m + 1
                else:
                    hi_ = m
            else:
                v = sum(max(0, m - L) for L in lags)
                if v < target:
                    lo_ = m + 1
                else:
                    hi_ = m
        return (hi_ // 8) * 8

    layers = []
    col = 0
    for p in range(3):
        per_lay = []
        for lay in range(3):
            lags = lag_sets[p][lay]
            lcols = sum(S - L for L in lags)
            b1 = find_b(lags, f_pool * lcols, True)
            b0 = find_b(lags, (1.0 - f_pool - f_scal) * lcols, False)
            b0 = min(b0, b1)
            scol, bcol = col, col + 1
            col += 2
            ops = []
            for L in lags:
                d_hi = max(L, b0)
                s_lo, s_hi = max(L, b0), max(L, b1)
                p_lo = max(L, b1)
                if S - p_lo < 384:  # pool seg too small: fold into scal seg
                    s_hi = S
                    p_lo = S
                ops.append((L, col, d_hi, s_lo, s_hi, p_lo))
                col += 1
            per_lay.append(dict(scol=scol, bcol=bcol, ops=ops))
        layers.append(per_lay)
    return layers, col


CONV_SCHED, CONV_NCOL = _conv_sched()


def _build_bass(reps=1, sim=False):
    import concourse.bacc as bacc
    import concourse.mybir as mybir
    import concourse.tile as tile

    f32 = mybir.dt.float32
    f16 = mybir.dt.float16
    AF = mybir.ActivationFunctionType
    OP = mybir.AluOpType

    nc = bacc.Bacc("TRN2", target_bir_lowering=False, debug=False,
                   num_devices=N_CORES)

    def din(name, shape, dt=f32):
        return nc.dram_tensor(name, shape, dt, kind="ExternalInput").ap()

    xT_d = din("xT", [HIDDEN, S], f16)
    wgT_d = din("wgT", [HIDDEN, HIDDEN], f16)
    rT_d = din("rT", [HIDDEN, 8], f16)
    rb_d = din("rb", [8, 1])
    csc_d = din("conv_sc", [128, CONV_NCOL])
    qbd_d = din("mem_qbd", [128, 128], f16)
    kvg_d = din("mem_kvg", [128, 386], f16)
    gbb_d = din("mem_gb_bc", [128, 2])
    wot_d = din("mem_WoT", [128, 256], f16)
    ones_d = din("ones64", [128, 64])
    eind_d = din("E_ind", [8, 384], f16)
    mgT_d = din("mixgT", [HIDDEN, HIDDEN], f16)
    mgb_d = din("mixgb", [HIDDEN, 1])
    mxT_d = din("mixT", [HIDDEN, HIDDEN], f16)
    mxb_d = din("mixb_bc", [128, HIDDEN])
    y_d = nc.dram_tensor("y", [B * TOK, HIDDEN], f16, kind="ExternalOutput").ap()

    with tile.TileContext(nc) as tc:
        with (
            tc.tile_pool(name="const", bufs=1) as constp,
            tc.tile_pool(name="main", bufs=1) as mainp,
            tc.tile_pool(name="xt", bufs=2) as xtp,
            tc.tile_pool(name="tmp", bufs=3) as tmpp,
            tc.tile_pool(name="ps", bufs=2, space="PSUM") as psp,
            tc.tile_pool(name="dram", bufs=1, space="DRAM") as dramp,
        ):
            # ---- resident weights: phase-1-critical on sync queue, the
            # rest on the Activation DGE queue so x streaming is unblocked
            wg_sb = [constp.tile([128, HIDDEN], f16, name=f"wg{i}") for i in range(NB)]
            rT_sb = [constp.tile([128, 8], f16, name=f"rt{i}") for i in range(NB)]
            for i in range(NB):
                nc.sync.dma_start(wg_sb[i][:], wgT_d[128 * i:128 * (i + 1), :])
                nc.sync.dma_start(rT_sb[i][:], rT_d[128 * i:128 * (i + 1), :])
            rb_sb = constp.tile([8, 1], f32, name="rb")
            nc.sync.dma_start(rb_sb[:], rb_d[:])
            csc_sb = constp.tile([128, CONV_NCOL], f32, name="csc")
            nc.scalar.dma_start(csc_sb[:], csc_d[:])
            qbd_sb = constp.tile([128, 128], f16, name="qbd")
            nc.scalar.dma_start(qbd_sb[:], qbd_d[:])
            kvg_sb = constp.tile([128, 386], f16, name="kvgw")
            nc.scalar.dma_start(kvg_sb[:], kvg_d[:])
            gbb_sb = constp.tile([128, 2], f32, name="gbb")
            nc.scalar.dma_start(gbb_sb[:], gbb_d[:])
            wot_sb = constp.tile([128, 256], f16, name="wot")
            ones_sb = constp.tile([128, 64], f32, name="ones")
            nc.scalar.dma_start(ones_sb[:], ones_d[:])
            eind_sb = constp.tile([8, 384], f16, name="eind")
            nc.scalar.dma_start(eind_sb[:], eind_d[:])
            mgT_sb = [constp.tile([128, HIDDEN], f16, name=f"mg{i}") for i in range(NB)]
            mxT_sb = [constp.tile([128, HIDDEN], f16, name=f"mx{i}") for i in range(NB)]
            mgb_sb = constp.tile([128, NB], f32, name="mgb")
            for i in range(NB):
                nc.scalar.dma_start(mgb_sb[:, i:i + 1], mgb_d[128 * i:128 * (i + 1), :])
            mxb_sb = constp.tile([128, HIDDEN], f32, name="mxb")

            max_sw = max((o[4] - o[3] for pl in CONV_SCHED for sch in pl
                          for o in sch["ops"]), default=0)
            max_pw = max((S - o[5] for pl in CONV_SCHED for sch in pl
                          for o in sch["ops"]), default=0)

            for _rep in range(reps):
              xg = [mainp.tile([128, S], f16, name=f"xg{p}", tag=f"xg{p}") for p in range(3)]
              C1 = [mainp.tile([128, S], f16, name=f"c1_{p}", tag=f"c1_{p}") for p in range(3)]
              C2m = mainp.tile([128, S], f16, name="c2m", tag="c2m")
              hw_sb = mainp.tile([8, S], f16, name="hww", tag="hww")

              # ======== Phase 1: gate matmul + SwiGLU + router ========
              with nc.named_scope("ph1_gate"):
               for ck in range(NCK):
                  cs = slice(512 * ck, 512 * (ck + 1))
                  xt = [xtp.tile([128, 512], f16, name=f"xt{i}", tag=f"xt{i}")
                        for i in range(NB)]
                  for i in range(NB):
                      eng = nc.sync if i % 2 == 0 else nc.scalar
                      eng.dma_start(xt[i][:], xT_d[128 * i:128 * (i + 1), cs])
                  ps_r = psp.tile([8, 512], f32, name="psr", tag="C", bufs=1)
                  for db in range(NB):
                      nc.tensor.matmul(ps_r[:], rT_sb[db][:], xt[db][:],
                                       start=(db == 0), stop=(db == NB - 1))
                  nc.scalar.activation(hw_sb[:, cs], ps_r[:], AF.Sigmoid,
                                       bias=rb_sb[:, 0:1], scale=1.0)
                  for pb in range(3):
                      ps_a = psp.tile([128, 512], f32, name="psa", tag="A")
                      ps_b = psp.tile([128, 512], f32, name="psb", tag="B")
                      for db in range(NB):
                          nc.tensor.matmul(
                              ps_a[:], wg_sb[db][:, 128 * pb:128 * (pb + 1)],
                              xt[db][:], start=(db == 0), stop=(db == NB - 1))
                      for db in range(NB):
                          nc.tensor.matmul(
                              ps_b[:],
                              wg_sb[db][:, 384 + 128 * pb:384 + 128 * (pb + 1)],
                              xt[db][:], start=(db == 0), stop=(db == NB - 1))
                      sig = tmpp.tile([128, 512], f32, name="sig", tag="sig")
                      nc.scalar.activation(sig[:], ps_b[:], AF.Sigmoid)
                      nc.vector.tensor_tensor(xg[pb][:, cs], ps_a[:], sig[:], OP.mult)

              if _rep == 0:
                  # deferred bulk weight loads: off the phase-1 critical path
                  nc.scalar.dma_start(wot_sb[:], wot_d[:])
                  for i in range(NB):
                      nc.scalar.dma_start(mgT_sb[i][:], mgT_d[128 * i:128 * (i + 1), :])
                      nc.scalar.dma_start(mxT_sb[i][:], mxT_d[128 * i:128 * (i + 1), :])
                  nc.scalar.dma_start(mxb_sb[:], mxb_d[:])

              # ======== neural memory scan (thunks; interleaved below) ====
              x_mem = xg[2]
              rd_ck = [mainp.tile([128, 512], f16, name=f"rdck{h}", tag=f"rdck{h}") for h in range(2)]
              mem_o = mainp.tile([128, S], f16, name="memo", tag="memo")
              M_a = mainp.tile([64, 128], f32, name="Ma", tag="Ma")
              M_b = mainp.tile([64, 128], f32, name="Mb", tag="Mb")
              nc.vector.memset(M_a[:], 0.0)
              nc.vector.memset(M_b[:], 0.0)

              def scan_thunks():
                  for ck4 in range(NCK):
                      # q projection for a full 512-chunk, both heads
                      cs4 = slice(512 * ck4, 512 * (ck4 + 1))
                      ps_qa = psp.tile([64, 512], f32, name="psqa", tag="D", bufs=1)
                      ps_qb = psp.tile([64, 512], f32, name="psqb", tag="E", bufs=1)
                      q_a = tmpp.tile([64, 512], f32, name="qa", tag="qa", bufs=2)
                      q_b = tmpp.tile([64, 512], f32, name="qb", tag="qb", bufs=2)

                      def q_proj(cs4=cs4, ps_qa=ps_qa, ps_qb=ps_qb, q_a=q_a, q_b=q_b):
                          nc.tensor.matmul(ps_qa[:], qbd_sb[:, 0:64],
                                           x_mem[:, cs4], start=True, stop=True)
                          nc.tensor.matmul(ps_qb[:], qbd_sb[:, 64:128],
                                           x_mem[:, cs4], start=True, stop=True)
                          nc.scalar.copy(q_a[:], ps_qa[:])
                          nc.scalar.copy(q_b[:], ps_qb[:])
                      yield q_proj
                      for bi in range(4):
                          blk = 4 * ck4 + bi
                          bs = slice(128 * blk, 128 * (blk + 1))
                          ps_rd = psp.tile([128, 256], f32, name="psrd", tag="F", bufs=1)
                          for half in range(2):
                              def step(blk=blk, half=half, bi=bi, ps_rd=ps_rd,
                                       q_a=q_a, q_b=q_b):
                                  c64 = slice(128 * blk + 64 * half,
                                              128 * blk + 64 * (half + 1))
                                  qc = slice(128 * bi + 64 * half,
                                             128 * bi + 64 * (half + 1))
                                  ps_kvg = psp.tile([64, 386], f32, name="pskvg",
                                                    tag="C", bufs=1)
                                  nc.tensor.matmul(ps_kvg[:], x_mem[:, c64],
                                                   kvg_sb[:], start=True, stop=True)
                                  g_sb = tmpp.tile([64, 2], f32, name="gsb", tag="gsb")
                                  for hh in range(2):
                                      nc.scalar.activation(
                                          g_sb[:, hh:hh + 1],
                                          ps_kvg[:, 384 + hh:385 + hh], AF.Sigmoid,
                                          bias=gbb_sb[0:64, hh:hh + 1], scale=1.0)
                                  kg_sb = tmpp.tile([64, 128], f16, name="kgsb", tag="kgsb")
                                  for hh in range(2):
                                      nc.vector.tensor_scalar(
                                          kg_sb[:, 64 * hh:64 * (hh + 1)],
                                          ps_kvg[:, 64 * hh:64 * (hh + 1)],
                                          g_sb[:, hh:hh + 1], None, OP.mult)
                                  v_sb = tmpp.tile([64, 256], f16, name="vsb", tag="vsb")
                                  nc.scalar.copy(v_sb[:], ps_kvg[:, 128:384])
                                  nc.tensor.matmul(
                                      ps_rd[:, 64 * half:64 * (half + 1)],
                                      M_a[:], q_a[:, qc], start=True, stop=True)
                                  nc.tensor.matmul(
                                      ps_rd[:, 128 + 64 * half:128 + 64 * (half + 1)],
                                      M_b[:], q_b[:, qc], start=True, stop=True)
                                  ps_g = psp.tile([64, 2], f32, name="psg", tag="E", bufs=1)
                                  nc.tensor.matmul(ps_g[:], ones_sb[0:64, :], g_sb[:],
                                                   start=True, stop=True)
                                  decay = tmpp.tile([64, 2], f32, name="decay", tag="decay")
                                  nc.scalar.activation(decay[:], ps_g[:], AF.Identity,
                                                       bias=1.0, scale=-1.0)
                                  ps_w = psp.tile([64, 256], f32, name="psw", tag="D", bufs=1)
                                  nc.tensor.matmul(ps_w[:, 0:128], kg_sb[:, 0:64],
                                                   v_sb[:, 0:128], start=True, stop=True)
                                  nc.tensor.matmul(ps_w[:, 128:256], kg_sb[:, 64:128],
                                                   v_sb[:, 128:256], start=True, stop=True)
                                  nc.vector.scalar_tensor_tensor(
                                      M_a[:], M_a[:], decay[:, 0:1], ps_w[:, 0:128],
                                      OP.mult, OP.add)
                                  nc.vector.scalar_tensor_tensor(
                                      M_b[:], M_b[:], decay[:, 1:2], ps_w[:, 128:256],
                                      OP.mult, OP.add)
                              yield step

                          def evict(blk=blk, bi=bi, ps_rd=ps_rd):
                              cc = 128 * bi
                              for hh in range(2):
                                  nc.scalar.copy(rd_ck[hh][:, cc:cc + 128],
                                                 ps_rd[:, 128 * hh:128 * (hh + 1)])
                          yield evict
                      def wout(ck4=ck4, cs4=cs4):
                          ps_o = psp.tile([128, 512], f32, name="pso", tag="C", bufs=1)
                          nc.tensor.matmul(ps_o[:], wot_sb[:, 0:128], rd_ck[0][:],
                                           start=True, stop=False)
                          nc.tensor.matmul(ps_o[:], wot_sb[:, 128:256], rd_ck[1][:],
                                           start=False, stop=True)
                          nc.scalar.copy(mem_o[:, cs4], ps_o[:])
                      yield wout

              # ======== conv thunks (two column waves) ========
              def chain_tiles(p):
                  if p < 2:
                      return [(xg[p], C1[p]), (C1[p], xg[p]), (xg[p], C1[p])]
                  return [(xg[2], C1[2]), (C1[2], C2m), (C2m, C1[2])]

              ctrs = {"ct": 0, "pt": 0}

              # wave-B of layer l reads layer-(l-1) output at cols < HS that
              # wave-A of the clobbering layer overwrites (ping-pong tile
              # reuse).  Snapshot the max-lag-wide boundary region before the
              # clobber; wave-B reads split at HS between snapshot and live.
              def laymax(p, lay):
                  return min(max((o[0] for o in CONV_SCHED[p][lay]["ops"]),
                                 default=0), HS)
              # snap_x[p]: original xg[p] (clobbered by lay1-A dst, p<2)
              # snap_0[p]: lay0 output (clobbered by lay2-A dst)
              snW_x = [laymax(p, 0) if p < 2 else 0 for p in range(3)]
              snW_0 = [laymax(p, 1) for p in range(3)]
              snap_x = [mainp.tile([128, max(snW_x[p], 8)], f16,
                                   name=f"snx{p}", tag=f"snx{p}")
                        for p in range(3)]
              snap_0 = [mainp.tile([128, max(snW_0[p], 8)], f16,
                                   name=f"sn0{p}", tag=f"sn0{p}")
                        for p in range(3)]

              def conv_thunks(wave):
                  w_lo, w_hi = (0, HS) if wave == 0 else (HS, S)
                  for lay in range(3):
                      for p in range(3):
                          src, dst = chain_tiles(p)[lay]
                          sch = CONV_SCHED[p][lay]
                          # wave-A: snapshot what this layer's dst clobbers
                          snap = None
                          if wave == 0:
                              if lay == 1 and p < 2 and snW_x[p] > 0:
                                  snap = (snap_x[p], snW_x[p], dst)
                              elif lay == 2 and snW_0[p] > 0:
                                  # lay2 dst holds lay0 output (= lay1 src)
                                  snap = (snap_0[p], snW_0[p], dst)
                          # wave-B: which snapshot replaces sub-HS src reads
                          rd_snap = None
                          if wave == 1:
                              if lay == 0 and p < 2 and snW_x[p] > 0:
                                  rd_snap = (snap_x[p], snW_x[p])
                              elif lay == 1 and snW_0[p] > 0:
                                  rd_snap = (snap_0[p], snW_0[p])

                          def base(src=src, dst=dst, sch=sch, snap=snap):
                              if snap is not None:
                                  st, sw, stile = snap
                                  nc.vector.tensor_copy(st[:, 0:sw],
                                                        stile[:, HS - sw:HS])
                              nc.vector.tensor_scalar(
                                  dst[:, w_lo:w_hi], src[:, w_lo:w_hi],
                                  csc_sb[:, sch["scol"]:sch["scol"] + 1],
                                  csc_sb[:, sch["bcol"]:sch["bcol"] + 1],
                                  OP.mult, OP.add)
                          yield base

                          def seg_pieces(lo, hi, L, rd_snap):
                              # split [lo,hi) dst cols at src boundary HS
                              if hi <= lo:
                                  return
                              if rd_snap is None or lo - L >= HS:
                                  yield (None, lo, hi)
                                  return
                              cut = min(hi, HS + L)
                              yield (rd_snap, lo, cut)
                              if hi > cut:
                                  yield (None, cut, hi)

                          for (L, cw, d_hi, s_lo, s_hi, p_lo) in sch["ops"]:
                              dl, dh = max(L, w_lo), min(d_hi, w_hi)
                              sl, sh = max(s_lo, w_lo), min(s_hi, w_hi)
                              pl, ph_ = max(p_lo, w_lo), w_hi
                              wcol = csc_sb[:, cw:cw + 1]

                              def tap(src=src, dst=dst, wcol=wcol, L=L, dl=dl,
                                      dh=dh, sl=sl, sh=sh, pl=pl, ph_=ph_,
                                      rd_snap=rd_snap):
                                  def sview(piece, lo, hi):
                                      rs, _, _ = piece
                                      if rs is None:
                                          return src[:, lo - L:hi - L]
                                      st, sw = rs
                                      off = HS - sw
                                      return st[:, lo - L - off:hi - L - off]
                                  for piece in seg_pieces(pl, ph_, L, rd_snap):
                                      _, lo, hi = piece
                                      pw = hi - lo
                                      ptmp = tmpp.tile([128, max_pw], f16,
                                                       name="pt",
                                                       tag=f"pt{ctrs['pt'] % 2}",
                                                       bufs=1)
                                      ctrs['pt'] += 1
                                      nc.gpsimd.tensor_scalar(
                                          ptmp[:, 0:pw], sview(piece, lo, hi),
                                          wcol, None, OP.mult)
                                      nc.gpsimd.tensor_tensor(
                                          dst[:, lo:hi], dst[:, lo:hi],
                                          ptmp[:, 0:pw], OP.add)
                                  for piece in seg_pieces(sl, sh, L, rd_snap):
                                      _, lo, hi = piece
                                      w = hi - lo
                                      tmp = tmpp.tile([128, max_sw], f16,
                                                      name="ct",
                                                      tag=f"ct{ctrs['ct'] % 3}",
                                                      bufs=1)
                                      ctrs['ct'] += 1
                                      nc.scalar.activation(
                                          tmp[:, 0:w], sview(piece, lo, hi),
                                          AF.Identity, bias=0.0, scale=wcol)
                                      nc.vector.tensor_tensor(
                                          dst[:, lo:hi], dst[:, lo:hi],
                                          tmp[:, 0:w], OP.add)
                                  for piece in seg_pieces(dl, dh, L, rd_snap):
                                      _, lo, hi = piece
                                      nc.vector.scalar_tensor_tensor(
                                          dst[:, lo:hi], sview(piece, lo, hi),
                                          wcol, dst[:, lo:hi], OP.mult, OP.add)
                              yield tap

              # ======== ph5: memory add + head weights (per wave) ========
              def ph5(wave):
                  for ck in range(4 * wave, 4 * wave + 4):
                      cs = slice(512 * ck, 512 * (ck + 1))
                      nc.vector.tensor_tensor(C1[2][:, cs], C1[2][:, cs],
                                              mem_o[:, cs], OP.add)
                  for p in range(3):
                      for ck in range(4 * wave, 4 * wave + 4):
                          cs = slice(512 * ck, 512 * (ck + 1))
                          ps_h = psp.tile([128, 512], f32, name="psh", tag="A")
                          nc.tensor.matmul(ps_h[:], eind_sb[:, 128 * p:128 * (p + 1)],
                                           hw_sb[:, cs], start=True, stop=True)
                          nc.vector.tensor_tensor(C1[p][:, cs], C1[p][:, cs],
                                                  ps_h[:], OP.mult)

              # ======== a2a wave: bounce out, collective, gather hT ========
              hT = [[mainp.tile([128, B * SUB], f16, name=f"ht{w}_{i}",
                               tag=f"ht{w}_{i}") for i in range(NB)]
                    for w in range(2)]

              def a2a(wave):
                  b_in = dramp.tile([N_CORES * 384, SUB], f16, name=f"bin{wave}",
                                    tag=f"bin{wave}")
                  b_out = dramp.tile([N_CORES * 384, SUB], f16, name=f"bout{wave}",
                                     tag=f"bout{wave}")
                  for j in range(N_CORES):
                      ts = slice(HS * wave + SUB * j, HS * wave + SUB * (j + 1))
                      for p in range(3):
                          nc.sync.dma_start(
                              b_in[384 * j + 128 * p:384 * j + 128 * (p + 1), :],
                              C1[p][:, ts])
                  if sim:
                      nc.sync.dma_start(b_out[:], b_in[:])
                  else:
                      import concourse.mybir as mybir
                      nc.gpsimd.collective_compute(
                          "AllToAll", mybir.AluOpType.bypass,
                          replica_groups=[list(range(N_CORES))],
                          ins=[b_in[:].opt()], outs=[b_out[:].opt()])
                  for b in range(B):
                      for fb in range(NB):
                          src_core = 2 * b + (1 if fb >= 3 else 0)
                          r0 = 384 * src_core + 128 * (fb % 3)
                          nc.sync.dma_start(hT[wave][fb][:, SUB * b:SUB * (b + 1)],
                                            b_out[r0:r0 + 128, :])

              # ======== ph7: mixing for one wave's sub-windows ========
              def ph7_thunks(wave):
                  ht = hT[wave]
                  for b in range(B):
                    def mixg_b(b=b):
                      cs = slice(SUB * b, SUB * (b + 1))
                      sigs = []
                      for fb in range(NB):
                          ps_pre = psp.tile([128, 256], f32, name="pre", tag="A")
                          for db in range(NB):
                              nc.tensor.matmul(ps_pre[:],
                                               mgT_sb[db][:, 128 * fb:128 * (fb + 1)],
                                               ht[db][:, cs], start=(db == 0),
                                               stop=(db == NB - 1))
                          sg = tmpp.tile([128, 256], f16, name=f"msig{fb}",
                                         tag=f"msig{fb}")
                          nc.scalar.activation(sg[:], ps_pre[:], AF.Sigmoid,
                                               bias=mgb_sb[:, fb:fb + 1], scale=1.0)
                          sigs.append(sg)
                      for fb in range(NB):
                          nc.vector.tensor_tensor(ht[fb][:, cs], ht[fb][:, cs],
                                                  sigs[fb][:], OP.mult)
                    yield mixg_b
                    def mix_b(b=b):
                      cs = slice(SUB * b, SUB * (b + 1))
                      for tb in range(2):
                          tr = slice(SUB * b + 128 * tb, SUB * b + 128 * (tb + 1))
                          yr = 512 * b + 256 * wave + 128 * tb
                          for half in range(2):
                              ps_y = psp.tile([128, 384], f32, name="psy",
                                              tag=("B" if half == 0 else "C"),
                                              bufs=(2 if half == 0 else 1))
                              for fb in range(NB):
                                  nc.tensor.matmul(
                                      ps_y[:], ht[fb][:, tr],
                                      mxT_sb[fb][:, 384 * half:384 * (half + 1)],
                                      start=(fb == 0), stop=(fb == NB - 1))
                              y_sb = tmpp.tile([128, 384], f16, name="ysb",
                                               tag=f"ysb{half}")
                              nc.vector.tensor_tensor(
                                  y_sb[:], ps_y[:],
                                  mxb_sb[:, 384 * half:384 * (half + 1)], OP.add)
                              nc.sync.dma_start(
                                  y_d[yr:yr + 128,
                                      384 * half:384 * (half + 1)],
                                  y_sb[:])
                    yield mix_b

              # ======== drive: interleave scan into conv wave A ========
              with nc.named_scope("ph3_convA"):
                  convA = list(conv_thunks(0))
                  scan = list(scan_thunks())
                  front = 15
                  for t in convA[:front]:
                      t()
                  rest = convA[front:]
                  si, n_s, n_c = 0, len(scan), len(rest)
                  for i, t in enumerate(rest):
                      t()
                      want = (i + 1) * n_s // max(n_c, 1)
                      while si < min(want, n_s):
                          scan[si]()
                          si += 1
                  while si < n_s:
                      scan[si]()
                      si += 1
              with nc.named_scope("ph5_A"):
                  ph5(0)
              with nc.named_scope("a2a_A"):
                  a2a(0)
              with nc.named_scope("ph3_convB"):
                  convB = list(conv_thunks(1))
                  mixA = list(ph7_thunks(0))
                  frontB = max(len(convB) // 3, 1)
                  for t in convB[:frontB]:
                      t()
                  restB = convB[frontB:]
                  mi, n_m, n_cb = 0, len(mixA), len(restB)
                  for i, t in enumerate(restB):
                      t()
                      want = (i + 1) * n_m // max(n_cb, 1)
                      while mi < min(want, n_m):
                          mixA[mi]()
                          mi += 1
                  while mi < n_m:
                      mixA[mi]()
                      mi += 1
              with nc.named_scope("ph5_B"):
                  ph5(1)
              with nc.named_scope("a2a_B"):
                  a2a(1)
              with nc.named_scope("ph7_B"):
                  for t in ph7_thunks(1):
                      t()

    nc.compile()
    return nc


def _prep_core_inputs(core, inp):
    b, g = core // 2, core % 2
    heads = GROUPS[g]
    f32, f16 = np.float32, np.float16

    x = np.asarray(inp["x"], f32)[b]
    gate_w = np.asarray(inp["gate_w"], f32)
    rows_xg = np.concatenate([np.arange(64 * h, 64 * h + 64) for h in heads])
    W_c = np.concatenate([gate_w[rows_xg], gate_w[768 + rows_xg]], axis=0)

    rT = np.zeros((HIDDEN, 8), f32)
    rT[:, :6] = np.asarray(inp["router_w"], f32)[heads].T
    rb = np.zeros((8, 1), f32)
    rb[:6, 0] = np.asarray(inp["router_b"], f32)[heads]

    conv_w = np.asarray(inp["conv_w"], f32)
    conv_b = np.asarray(inp["conv_b"], f32)
    csc = np.zeros((128, CONV_NCOL), f32)
    for p in range(3):
        for lay in range(3):
            sch = CONV_SCHED[p][lay]
            for hh in range(2):
                head = heads[2 * p + hh]
                rows = slice(64 * hh, 64 * (hh + 1))
                csc[rows, sch["scol"]] = 1.0 + conv_w[head, lay, :, 3]
                csc[rows, sch["bcol"]] = conv_b[head, lay, :]
                d = DILATIONS[head][lay]
                for (L, cw, *_rest) in sch["ops"]:
                    if L % d == 0 and 1 <= L // d <= 3:
                        csc[rows, cw] = conv_w[head, lay, :, 3 - L // d]

    ma, mb = heads[4], heads[5]
    ia, ib = MEM_HEADS.index(ma), MEM_HEADS.index(mb)
    Wq = np.asarray(inp["mem_Wq"], f32)
    Wk = np.asarray(inp["mem_Wk"], f32)
    Wv = np.asarray(inp["mem_Wv"], f32)
    Wgw = np.asarray(inp["mem_Wg_w"], f32)
    Wgb = np.asarray(inp["mem_Wg_b"], f32)
    Wo = np.asarray(inp["mem_Wout"], f32)

    qbd = np.zeros((128, 128), f32)
    qbd[0:64, 0:64] = Wq[ia].T
    qbd[64:128, 64:128] = Wq[ib].T
    kvg = np.zeros((128, 386), f32)
    kvg[0:64, 0:64] = Wk[ia].T
    kvg[64:128, 64:128] = Wk[ib].T
    kvg[0:64, 128:256] = Wv[ia].T
    kvg[64:128, 256:384] = Wv[ib].T
    kvg[0:64, 384] = Wgw[ia, 0]
    kvg[64:128, 385] = Wgw[ib, 0]
    gbb = np.zeros((128, 2), f32)
    gbb[:, 0] = Wgb[ia, 0]
    gbb[:, 1] = Wgb[ib, 0]
    wot = np.zeros((128, 256), f32)
    wot[:, 0:64] = Wo[ia].T           # head-a rows 0:64 of stacked out
    wot[:, 128 + 64:256] = Wo[ib].T   # head-b rows 64:128 of stacked out

    eind = np.zeros((8, 384), f32)
    for p in range(3):
        eind[2 * p, 128 * p:128 * p + 64] = 1.0
        eind[2 * p + 1, 128 * p + 64:128 * (p + 1)] = 1.0

    pf = np.concatenate([np.arange(64 * h, 64 * h + 64) for h in PERM_HEADS])
    mixg_w = np.asarray(inp["mixg_w"], f32)
    mix_w = np.asarray(inp["mix_w"], f32)

    return {
        "xT": np.ascontiguousarray(x.T).astype(f16),
        "wgT": np.ascontiguousarray(W_c.T).astype(f16),
        "rT": rT.astype(f16), "rb": rb, "conv_sc": csc,
        "mem_qbd": qbd.astype(f16), "mem_kvg": kvg.astype(f16),
        "mem_gb_bc": gbb, "mem_WoT": wot.astype(f16),
        "ones64": np.full((128, 64), 1.0 / 64.0, f32),
        "E_ind": eind.astype(f16),
        "mixgT": np.ascontiguousarray(mixg_w[np.ix_(pf, pf)].T).astype(f16),
        "mixgb": np.asarray(inp["mixg_b"], f32)[pf].reshape(HIDDEN, 1).copy(),
        "mixT": np.ascontiguousarray(mix_w[:, pf].T).astype(f16),
        "mixb_bc": np.tile(np.asarray(inp["mix_b"], f32)[None, :], (128, 1)),
    }


def prep_in_maps(inputs):
    return [_prep_core_inputs(c, inputs) for c in range(N_CORES)]


def get_bass():
    if "nc" not in _CACHE:
        _CACHE["nc"] = _build_bass()
    return _CACHE["nc"]


def assemble(results):
    out = np.zeros((B, S, HIDDEN), np.float32)
    for j in range(N_CORES):
        y = results[j]["y"].astype(np.float32)   # [B*TOK, H] f16
        for b in range(B):
            for w in range(2):
                rows = y[512 * b + 256 * w:512 * b + 256 * w + 256]
                out[b, HS * w + SUB * j:HS * w + SUB * (j + 1), :] = rows
    return out


def kernel(**inputs):
    from concourse import bass_utils
    nc = get_bass()
    in_maps = prep_in_maps(inputs)
    res = bass_utils.run_bass_kernel_spmd(nc, in_maps,
                                          core_ids=list(range(N_CORES)))
    return assemble(res.results)
